# revision 1
# baseline (speedup 1.0000x reference)
"""Trainium2 Bass kernel for nn_DipoleEnergyLean (gnn_message_passing).

Strategy (8 NeuronCores, SPMD):
  - Atoms sharded by molecule: core c owns molecules [32c,32c+32) = atoms
    [3072c, 3072c+3072). Weights replicated.
  - Per-species MLP dispatch done with mask-multiply in a feature-major
    (transposed) layout [F, atoms] so everything is plain matmuls.
  - Edge aggregation (segment_sum over 1.57M contributions with global
    random indices): contributions are sorted by destination atom on the
    host (index metadata only), sharded by destination core. Each core
    computes its nbr rows, AllGathers the full nbr table to HBM, then
    per 128-contribution tile:
      dma_gather 128 source rows -> msg [128, 128f32]
      one DVE op builds a decay-weighted one-hot OH[c,a] = (iota==dloc)*w
      PE matmul psum[128 atoms, F] += OH.T @ msg
    accumulating T tiles per 128-atom destination block in PSUM.
  - Final per-species head + per-molecule charge redistribution on-chip.

The harness calls kernel(**inputs) with the full unsharded arrays; this
file shards on the host, runs the SPMD Bass kernel on cores 0-7 via
run_bass_kernel_spmd, and reassembles the full output.
"""

import numpy as np

# ---------------------------------------------------------------- sizes
B, A, D, H, F = 256, 96, 384, 192, 96
S = 4
N = B * A                 # 24576 atoms
E = N * 32                # 786432 edges
CUTOFF = 5.2
NCORE = 8
AC = N // NCORE           # 3072 atoms / core
MOL = B // NCORE          # 32 molecules / core
BLK = 128                 # destination block (atoms)
NBLK = N // BLK           # 192 global blocks
NBLK_C = AC // BLK        # 24 blocks / core
NTA = AC // 128           # 24 atom tiles / core
FP = 128                  # padded feature row (512 B) for gather
NCH = AC // 512           # 6 atom chunks of 512


# ---------------------------------------------------------------- host prep
def _preprocess_edges(atom_index12, distances):
    """Sort contributions by destination; pad each 128-atom destination
    block to a uniform T tiles of 128 contributions (same T for all cores
    so one compiled program serves all 8)."""
    i0 = atom_index12[0].astype(np.int64)
    i1 = atom_index12[1].astype(np.int64)
    dest = np.concatenate([i0, i1])
    src = np.concatenate([i1, i0])
    dd = np.concatenate([distances, distances]).astype(np.float32)

    order = np.argsort(dest, kind="stable")
    dest_s = dest[order]
    src_s = src[order]
    dd_s = dd[order]

    counts = np.bincount(dest_s // BLK, minlength=NBLK)
    T = int(np.ceil(counts.max() / 128))
    K = NBLK_C * T * 128

    starts = np.zeros(NBLK + 1, np.int64)
    np.cumsum(counts, out=starts[1:])

    eidx = np.zeros((NCORE, K), np.int16)
    dloc = np.full((NCORE, K), -1.0, np.float32)
    dval = np.zeros((NCORE, K), np.float32)
    for c in range(NCORE):
        for bb in range(NBLK_C):
            g = c * NBLK_C + bb
            s0, s1 = starts[g], starts[g + 1]
            n = s1 - s0
            o = bb * T * 128
            eidx[c, o:o + n] = src_s[s0:s1].astype(np.int16)
            dloc[c, o:o + n] = (dest_s[s0:s1] - g * BLK).astype(np.float32)
            dval[c, o:o + n] = dd_s[s0:s1]
    return T, K, eidx, dloc, dval


# ---------------------------------------------------------------- device kernel
_CACHE = {}


_STAGE = 3
_NBLOCKS = 1
_REPEAT = 1
_EDGE_REPEAT = 1


def _build(T, K, dp2, df2):
    import concourse.bass as bass
    import concourse.bacc as bacc
    import concourse.mybir as mybir
    import concourse.tile as tile
    from concourse.masks import make_identity

    f32 = mybir.dt.float32
    i32 = mybir.dt.int32
    i16 = mybir.dt.int16
    AF = mybir.ActivationFunctionType
    OP = mybir.AluOpType
    KT = K // 128             # contribution tiles total

    nc = bacc.Bacc("TRN2", target_bir_lowering=False, num_devices=NCORE,
                   num_swdge_queues=4)

    x_in = nc.dram_tensor("x", [AC, D], f32, kind="ExternalInput")
    sp_in = nc.dram_tensor("species", [1, AC], i32, kind="ExternalInput")
    tc_in = nc.dram_tensor("tcharge", [1, MOL], f32, kind="ExternalInput")
    w1_in = nc.dram_tensor("W1", [S, D, H], f32, kind="ExternalInput")
    w2_in = nc.dram_tensor("W2", [S, H, F], f32, kind="ExternalInput")
    wn_in = nc.dram_tensor("Wn", [S, F, F], f32, kind="ExternalInput")
    wf_in = nc.dram_tensor("Wf", [S, 2 * F, 1], f32, kind="ExternalInput")
    eidx_in = nc.dram_tensor("eidx", [128, KT], i32, kind="ExternalInput")
    dloc_in = nc.dram_tensor("dloc", [128, KT], f32, kind="ExternalInput")
    dval_in = nc.dram_tensor("dval", [128, KT], f32, kind="ExternalInput")
    out_t = nc.dram_tensor("out", [2, AC], f32, kind="ExternalOutput")

    nbr_local = nc.dram_tensor("nbr_local", [AC, FP], f32)
    nbr_full_sh = nc.dram_tensor("nbr_full_sh", [N, FP], f32, addr_space="Shared")
    nbr_full = nc.dram_tensor("nbr_full", [N, FP], f32)

    with tile.TileContext(nc) as tc:
        # ---------------- persistent tiles (kept for the whole kernel)
        _keep = []
        def _single(shape, dtype, name):
            t, free = tc.tile(shape, dtype, name=name)
            _keep.append(free)
            return t
        ident = _single([128, 128], f32, "ident")
        make_identity(nc, ident[:, :])
        iota_row = _single([128, 128], f32, "iota_row")
        internalT = _single([F, AC], f32, "internalT")
        mergedT = _single([F, AC], f32, "mergedT")
        eqs = _single([S, AC], f32, "eqs")
        spf = _single([1, AC], f32, "spf")
        ones96 = _single([1, 96], f32, "ones96")
        nc.vector.memset(ones96[:, :], 1.0)

        ones4r = _single([1, S], f32, "ones4r")
        nc.vector.memset(ones4r[:, :], 1.0)
        with tc.tile_pool(name="init_pool", bufs=1) as ip, \
             tc.tile_pool(name="init_psum", bufs=2, space="PSUM") as ipp:
            iota_i = ip.tile([128, 128], i32, name="iota_i")
            nc.gpsimd.iota(iota_i[:, :], pattern=[[1, 128]], base=0,
                           channel_multiplier=0)
            nc.vector.tensor_copy(iota_row[:, :], iota_i[:, :])
            sp_i = ip.tile([1, AC], i32, name="sp_i")
            nc.sync.dma_start(sp_i[:, :], sp_in[:, :])
            nc.vector.tensor_copy(spf[:, :], sp_i[:, :])
            # eqs[s, a] = (species[a] == s): broadcast species to 4
            # partitions via K=1 matmul, compare against per-partition iota
            svec_i = ip.tile([S, 1], i32, name="svec_i")
            nc.gpsimd.iota(svec_i[:, :], pattern=[[0, 1]], base=0,
                           channel_multiplier=1)
            svec = ip.tile([S, 1], f32, name="svec")
            nc.vector.tensor_copy(svec[:, :], svec_i[:, :])
            for cch in range(NCH):
                sl = slice(cch * 512, (cch + 1) * 512)
                sp4 = ipp.tile([S, 512], f32, tag="sp4", name="sp4")
                nc.tensor.matmul(sp4[:, :], ones4r[:, :], spf[:, sl],
                                 start=True, stop=True)
                nc.vector.tensor_scalar(
                    out=eqs[:, sl], in0=sp4[:, :], scalar1=svec[:, :],
                    scalar2=None, op0=OP.is_equal)

        tc.strict_bb_all_engine_barrier()

        # ---------------- phase 1: per-species MLP (transposed layout)
        for _rep in range(_REPEAT):
            _run_phases(nc, tc, mybir, bass, tile, T, K, dp2, df2,
                        ident, iota_row, internalT, mergedT, eqs, spf, ones96,
                        x_in, tc_in, w1_in, w2_in, wn_in, wf_in, eidx_in,
                        dloc_in, dval_in, out_t, nbr_local, nbr_full_sh,
                        nbr_full)

        for free in reversed(_keep):
            free()

    nc.compile()
    return nc


def _run_phases(nc, tc, mybir, bass, tile, T, K, dp2, df2,
                ident, iota_row, internalT, mergedT, eqs, spf, ones96,
                x_in, tc_in, w1_in, w2_in, wn_in, wf_in, eidx_in,
                dloc_in, dval_in, out_t, nbr_local, nbr_full_sh, nbr_full):
        f32 = mybir.dt.float32
        i32 = mybir.dt.int32
        i16 = mybir.dt.int16
        AF = mybir.ActivationFunctionType
        OP = mybir.AluOpType
        KT = K // 128

        # ---------------- phase 1: per-species MLP (transposed layout)
        with tc.tile_pool(name="mlp_sbuf", bufs=1) as mp, \
             tc.tile_pool(name="mlp_work", bufs=3) as wk:

            # x -> xT (3 k-tiles of [128, AC])
            xT = [mp.tile([128, AC], f32, tag=f"xT{k}", name=f"xT{k}")
                  for k in range(3)]
            with tc.tile_pool(name="tr_psum", bufs=4, space="PSUM") as trp:
                for t in range(NTA):
                    xa = wk.tile([128, D], f32, tag="xa", name="xa")
                    nc.sync.dma_start(xa[:, :], x_in[t * 128:(t + 1) * 128, :])
                    for k in range(3):
                        tp = trp.tile([128, 128], f32, tag="xtp", name="xtp")
                        nc.tensor.transpose(tp[:, :],
                                            xa[:, k * 128:(k + 1) * 128],
                                            ident[:, :])
                        nc.vector.tensor_copy(
                            xT[k][:, t * 128:(t + 1) * 128], tp[:, :])

            # weights
            w1t = [[mp.tile([128, H], f32, tag=f"w1_{s}_{k}", name=f"w1_{s}_{k}")
                    for k in range(3)] for s in range(S)]
            w2t = [[mp.tile([96, F], f32, tag=f"w2_{s}_{k}", name=f"w2_{s}_{k}")
                    for k in range(2)] for s in range(S)]
            wnt = [mp.tile([F, F], f32, tag=f"wn_{s}", name=f"wn_{s}")
                   for s in range(S)]
            for s in range(S):
                for k in range(3):
                    nc.sync.dma_start(w1t[s][k][:, :],
                                      w1_in[s, k * 128:(k + 1) * 128, :])
                for k in range(2):
                    nc.sync.dma_start(w2t[s][k][:, :],
                                      w2_in[s, k * 96:(k + 1) * 96, :])
                nc.sync.dma_start(wnt[s][:, :], wn_in[s, :, :])

            nbrT = mp.tile([F, AC], f32, tag="nbrT", name="nbrT")

            with tc.tile_pool(name="mlp_psum", bufs=2, space="PSUM") as pp, \
                 tc.tile_pool(name="mlp_psum1", bufs=1, space="PSUM") as pp1:
                for cch in range(NCH):
                    sl = slice(cch * 512, (cch + 1) * 512)
                    pn = pp1.tile([F, 512], f32, tag="pn", name="pn")
                    for s in range(S):
                        # species mask chunk, broadcast to 96 partitions
                        eq_c = wk.tile([1, 512], f32, tag="eq_c", name="eq_c")
                        nc.vector.tensor_scalar(
                            out=eq_c[:, :], in0=spf[:, sl], scalar1=float(s),
                            scalar2=None, op0=OP.is_equal)
                        mps = pp.tile([F, 512], f32, tag="mps", name="mps")
                        nc.tensor.matmul(mps[:, :], ones96[:, :96], eq_c[:, :],
                                         start=True, stop=True)
                        mask_c = wk.tile([F, 512], f32, tag="mask_c",
                                         name="mask_c")
                        nc.vector.tensor_copy(mask_c[:, :], mps[:, :])

                        ph0 = pp.tile([96, 512], f32, tag="ph0", name="ph0")
                        ph1 = pp.tile([96, 512], f32, tag="ph1", name="ph1")
                        for k in range(3):
                            nc.tensor.matmul(ph0[:, :], w1t[s][k][:, 0:96],
                                             xT[k][:, sl], start=(k == 0),
                                             stop=(k == 2))
                        for k in range(3):
                            nc.tensor.matmul(ph1[:, :], w1t[s][k][:, 96:192],
                                             xT[k][:, sl], start=(k == 0),
                                             stop=(k == 2))
                        h0 = wk.tile([96, 512], f32, tag="h0", name="h0")
                        h1 = wk.tile([96, 512], f32, tag="h1", name="h1")
                        nc.scalar.activation(h0[:, :], ph0[:, :],
                                             AF.Gelu_apprx_tanh)
                        nc.scalar.activation(h1[:, :], ph1[:, :],
                                             AF.Gelu_apprx_tanh)
                        pv = pp1.tile([F, 512], f32, tag="pv", name="pv")
                        nc.tensor.matmul(pv[:, :], w2t[s][0][:, :], h0[:, :],
                                         start=True, stop=False)
                        nc.tensor.matmul(pv[:, :], w2t[s][1][:, :], h1[:, :],
                                         start=False, stop=True)
                        if s == 0:
                            nc.vector.tensor_tensor(
                                out=internalT[:, sl], in0=pv[:, :],
                                in1=mask_c[:, :], op=OP.mult)
                            rhs = internalT[:, sl]
                        else:
                            tmp = wk.tile([F, 512], f32, tag="tmp", name="tmp")
                            nc.vector.tensor_tensor(
                                out=tmp[:, :], in0=pv[:, :],
                                in1=mask_c[:, :], op=OP.mult)
                            nc.vector.tensor_tensor(
                                out=internalT[:, sl], in0=internalT[:, sl],
                                in1=tmp[:, :], op=OP.add)
                            rhs = tmp[:, :]
                        nc.tensor.matmul(pn[:, :], wnt[s][:, :], rhs,
                                         start=(s == 0), stop=(s == S - 1))
                    nc.vector.tensor_copy(nbrT[:, sl], pn[:, :])

            # stage nbr rows (transposed back) and write the local table
            with tc.tile_pool(name="st_psum", bufs=2, space="PSUM") as stp, \
                 tc.tile_pool(name="st_work", bufs=3) as stw:
                nbr_rows = nbr_local.rearrange("(t p) e -> p t e", p=128)
                for t in range(NTA):
                    ntp = stp.tile([128, 96], f32, tag="ntp", name="ntp")
                    nc.tensor.transpose(ntp[:, :],
                                        nbrT[:, t * 128:(t + 1) * 128],
                                        ident[0:96, 0:96])
                    strow = stw.tile([128, FP], f32, tag="strow", name="strow")
                    nc.vector.memset(strow[:, 96:128], 0.0)
                    nc.vector.tensor_copy(strow[:, 0:96], ntp[:, :])
                    nc.sync.dma_start(nbr_rows[:, t, :], strow[:, :])

        tc.strict_bb_all_engine_barrier()

        # ---------------- phase 2: AllGather the nbr table
        if _STAGE >= 2:
            nc.gpsimd.collective_compute(
                "AllGather", mybir.AluOpType.bypass,
                replica_groups=[list(range(NCORE))],
                ins=[nbr_local[:, :]],
                outs=[nbr_full_sh[:, :]],
            )
            # gather source must be Local address space; bounce the table
            nc.sync.dma_start(nbr_full[:, :], nbr_full_sh[:, :])

        tc.strict_bb_all_engine_barrier()

        # ---------------- phase 3: edge aggregation
        with tc.tile_pool(name="edge_sbuf", bufs=1) as ep, \
             tc.tile_pool(name="edge_work", bufs=8) as ew, \
             tc.tile_pool(name="msg_pool", bufs=16) as mgp, \
             tc.tile_pool(name="edge_psum", bufs=4, space="PSUM") as epp:

            # per-tile gather indices: [p, j] = source row for tile j,
            # partition p (int32, one indirect DMA per 128-row tile)
            idxs = ep.tile([128, KT], i32, tag="idxs", name="idxs")
            nc.sync.dma_start(idxs[:, :], eidx_in[:, :])
            dloc = ep.tile([128, KT], f32, tag="dloc", name="dloc")
            nc.sync.dma_start(dloc[:, :], dloc_in[:, :])
            dv = ep.tile([128, KT], f32, tag="dv", name="dv")
            nc.sync.dma_start(dv[:, :], dval_in[:, :])

            # decay weights w = dp2 * exp(-df2*d) * smooth_cutoff(d)
            xx = ep.tile([128, KT], f32, tag="xx", name="xx")
            wgt = ep.tile([128, KT], f32, tag="wgt", name="wgt")
            sc1 = ep.tile([128, KT], f32, tag="sc1", name="sc1")
            nc.vector.tensor_scalar(out=xx[:, :], in0=dv[:, :], scalar1=CUTOFF,
                                    scalar2=-1.0 / CUTOFF, op0=OP.subtract,
                                    op1=OP.mult)
            nc.vector.tensor_scalar(out=xx[:, :], in0=xx[:, :], scalar1=0.0,
                                    scalar2=1.0, op0=OP.max, op1=OP.min)
            nc.vector.tensor_scalar(out=sc1[:, :], in0=xx[:, :], scalar1=6.0,
                                    scalar2=-15.0, op0=OP.mult, op1=OP.add)
            nc.vector.tensor_tensor(out=sc1[:, :], in0=sc1[:, :], in1=xx[:, :],
                                    op=OP.mult)
            nc.vector.tensor_scalar(out=sc1[:, :], in0=sc1[:, :], scalar1=10.0,
                                    scalar2=None, op0=OP.add)
            nc.vector.tensor_tensor(out=wgt[:, :], in0=xx[:, :], in1=xx[:, :],
                                    op=OP.mult)
            nc.vector.tensor_tensor(out=wgt[:, :], in0=wgt[:, :], in1=sc1[:, :],
                                    op=OP.mult)
            nc.vector.scalar_tensor_tensor(out=wgt[:, :], in0=wgt[:, :],
                                           scalar=float(dp2), in1=xx[:, :],
                                           op0=OP.mult, op1=OP.mult)
            nc.scalar.activation(sc1[:, :], dv[:, :], AF.Exp, scale=-float(df2))
            nc.vector.tensor_tensor(out=wgt[:, :], in0=wgt[:, :], in1=sc1[:, :],
                                    op=OP.mult)

            nblocks = NBLK_C if _STAGE >= 3 else (_NBLOCKS if _STAGE >= 2 else 0)
            if nblocks < NBLK_C:
                nc.vector.memset(mergedT[:, :], 0.0)
            for b in [bb for _er in range(_EDGE_REPEAT)
                      for bb in range(nblocks)]:
                pm = epp.tile([128, 96], f32, tag="pm", name="pm")
                for j in range(T):
                    col = b * T + j
                    msg = mgp.tile([128, FP], f32, tag="msg", name="msg")
                    nc.gpsimd.indirect_dma_start(
                        out=msg[:, :], out_offset=None,
                        in_=nbr_full[:, :],
                        in_offset=bass.IndirectOffsetOnAxis(
                            ap=idxs[:, col:col + 1], axis=0))
                    oh = ew.tile([128, 128], f32, tag="oh", name="oh")
                    nc.vector.tensor_scalar(
                        out=oh[:, :], in0=iota_row[:, :],
                        scalar1=dloc[:, col:col + 1],
                        scalar2=wgt[:, col:col + 1],
                        op0=OP.is_equal, op1=OP.mult)
                    nc.tensor.matmul(pm[:, :], oh[:, :], msg[:, 0:96],
                                     start=(j == 0), stop=(j == T - 1))
                ms = ew.tile([128, 96], f32, tag="ms", name="ms")
                nc.vector.tensor_copy(ms[:, :], pm[:, :])
                pt = epp.tile([96, 128], f32, tag="pt", name="pt")
                nc.tensor.transpose(pt[:, :], ms[:, :], ident[:, :])
                nc.vector.tensor_copy(mergedT[:, b * 128:(b + 1) * 128],
                                      pt[:, :])

        tc.strict_bb_all_engine_barrier()

        # ---------------- phase 4: final head + charge redistribution
        with tc.tile_pool(name="head_sbuf", bufs=1) as hp, \
             tc.tile_pool(name="head_work", bufs=3) as hw, \
             tc.tile_pool(name="head_psum", bufs=2, space="PSUM") as hpp:

            wfi = hp.tile([96, S], f32, tag="wfi", name="wfi")
            wfm = hp.tile([96, S], f32, tag="wfm", name="wfm")
            for s in range(S):
                nc.sync.dma_start(wfi[:, s:s + 1], wf_in[s, 0:96, 0:1])
                nc.sync.dma_start(wfm[:, s:s + 1], wf_in[s, 96:192, 0:1])

            pre = hp.tile([1, AC], f32, tag="pre", name="pre")
            chg = hp.tile([1, AC], f32, tag="chg", name="chg")
            ones4 = hp.tile([S, 1], f32, tag="ones4", name="ones4")
            nc.vector.memset(ones4[:, :], 1.0)

            for cch in range(NCH):
                sl = slice(cch * 512, (cch + 1) * 512)
                pp4 = hpp.tile([S, 512], f32, tag="pp4", name="pp4")
                nc.tensor.matmul(pp4[:, :], wfi[:, :], internalT[:, sl],
                                 start=True, stop=False)
                nc.tensor.matmul(pp4[:, :], wfm[:, :], mergedT[:, sl],
                                 start=False, stop=True)
                sel = hw.tile([S, 512], f32, tag="sel", name="sel")
                nc.vector.tensor_tensor(out=sel[:, :], in0=pp4[:, :],
                                        in1=eqs[:, sl], op=OP.mult)
                pr1 = hpp.tile([1, 512], f32, tag="pr1", name="pr1")
                nc.tensor.matmul(pr1[:, :], ones4[:, :], sel[:, :],
                                 start=True, stop=True)
                nc.vector.tensor_copy(pre[:, sl], pr1[:, :])

            # per-molecule redistribution
            tct = hp.tile([1, MOL], f32, tag="tct", name="tct")
            nc.sync.dma_start(tct[:, :], tc_in[:, :])
            msum = hp.tile([1, MOL], f32, tag="msum", name="msum")
            nc.vector.tensor_reduce(
                out=msum[:, :],
                in_=pre[:, :].rearrange("p (m a) -> p m a", a=A),
                axis=mybir.AxisListType.X, op=OP.add)
            adj = hp.tile([1, MOL], f32, tag="adj", name="adj")
            nc.vector.tensor_tensor(out=adj[:, :], in0=tct[:, :],
                                    in1=msum[:, :], op=OP.subtract)
            nc.vector.tensor_scalar(out=adj[:, :], in0=adj[:, :],
                                    scalar1=1.0 / A, scalar2=None, op0=OP.mult)
            nc.vector.tensor_tensor(
                out=chg[:, :].rearrange("p (m a) -> p m a", a=A),
                in0=pre[:, :].rearrange("p (m a) -> p m a", a=A),
                in1=adj[:, :].to_broadcast([1, MOL, A]),
                op=OP.add)

            nc.sync.dma_start(out_t[0:1, :], chg[:, :])
            nc.sync.dma_start(out_t[1:2, :], pre[:, :])


def _get_nc(T, K, dp2, df2):
    key = (T, K, round(float(dp2), 9), round(float(df2), 9))
    if key not in _CACHE:
        _CACHE[key] = _build(T, K, dp2, df2)
    return _CACHE[key]


# ---------------------------------------------------------------- entry point
def kernel(species, in_features, atom_index12, distances, total_charges,
           W1, W2, Wn, Wf, decay_prefactor, decay_factor, _trace=False):
    from concourse.bass_utils import run_bass_kernel_spmd

    species = np.asarray(species, np.int32)
    in_features = np.ascontiguousarray(np.asarray(in_features, np.float32))
    atom_index12 = np.asarray(atom_index12, np.int32)
    distances = np.asarray(distances, np.float32)
    total_charges = np.asarray(total_charges, np.float32)
    W1 = np.ascontiguousarray(np.asarray(W1, np.float32))
    W2 = np.ascontiguousarray(np.asarray(W2, np.float32))
    Wn = np.ascontiguousarray(np.asarray(Wn, np.float32))
    Wf = np.ascontiguousarray(np.asarray(Wf, np.float32))
    dp2 = float(np.asarray(decay_prefactor)) ** 2
    df2 = float(np.asarray(decay_factor)) ** 2

    T, K, eidx, dloc, dval = _preprocess_edges(atom_index12, distances)
    nc = _get_nc(T, K, dp2, df2)

    in_maps = []
    for c in range(NCORE):
        in_maps.append({
            "x": np.ascontiguousarray(
                in_features[c * MOL:(c + 1) * MOL].reshape(AC, D)),
            "species": np.ascontiguousarray(
                species[c * MOL:(c + 1) * MOL].reshape(1, AC)),
            "tcharge": np.ascontiguousarray(
                total_charges[c * MOL:(c + 1) * MOL].reshape(1, MOL)),
            "W1": W1, "W2": W2, "Wn": Wn, "Wf": Wf,
            "eidx": np.ascontiguousarray(
                eidx[c].astype(np.int32).reshape(K // 128, 128).T),
            "dloc": np.ascontiguousarray(dloc[c].reshape(K // 128, 128).T),
            "dval": np.ascontiguousarray(dval[c].reshape(K // 128, 128).T),
        })

    res = run_bass_kernel_spmd(nc, in_maps, core_ids=list(range(NCORE)),
                               trace=_trace)
    charges = np.empty((B, A), np.float32)
    precharges = np.empty((B, A), np.float32)
    for c in range(NCORE):
        o = res.results[c]["out"]
        charges[c * MOL:(c + 1) * MOL] = o[0].reshape(MOL, A)
        precharges[c * MOL:(c + 1) * MOL] = o[1].reshape(MOL, A)
    if _trace:
        kernel._last_results = res
    return species.reshape(B, A), charges, precharges



# revision 14
# speedup vs baseline: 1.3960x; 1.3960x over previous
"""Trainium2 Bass kernel for nn_DipoleEnergyLean (gnn_message_passing).

Strategy (8 NeuronCores, SPMD, bf16 compute / f32 accumulate):
  - Atoms sharded by molecule: core c owns molecules [32c,32c+32) = atoms
    [3072c, 3072c+3072). Weights replicated.
  - Per-species MLP via masked INPUTS accumulated in PSUM across species
    (gelu(0)=0 makes input-masking exact), feature-major layout.
  - Edge aggregation: contributions sorted by destination atom on the
    host and packed into "quad slots" (4 same-destination contributions
    per slot). Per 128-slot tile: one one-hot matmul scatters 512
    contributions. Gathers are batched ~GQ tiles (up to ~4.6K rows) per
    indirect DMA to amortize the ~1us SWDGE fixed cost. The nbr table is
    bf16 (256B rows) to halve gather traffic.
  - AllGather the bf16 nbr table, bounce Shared->Local split across all
    DMA queues, then per destination block accumulate T quad-tiles in
    PSUM via msgw^T @ onehot matmuls writing mergedT directly.
  - Final per-species head + per-molecule charge redistribution on-chip.

The harness calls kernel(**inputs) with the full unsharded arrays; this
file shards on the host, runs the SPMD Bass kernel on cores 0-7 via
run_bass_kernel_spmd, and reassembles the full output.
"""

import numpy as np

# ---------------------------------------------------------------- sizes
B, A, D, H, F = 256, 96, 384, 192, 96
S = 4
N = B * A                 # 24576 atoms
E = N * 32                # 786432 edges
CUTOFF = 5.2
NCORE = 8
AC = N // NCORE           # 3072 atoms / core
MOL = B // NCORE          # 32 molecules / core
BLK = 128                 # destination block (atoms)
NBLK = N // BLK           # 192 global blocks
NBLK_C = AC // BLK        # 24 blocks / core
NTA = AC // 128           # 24 atom tiles / core
NCH = AC // 512           # 6 atom chunks of 512
QM = 4                    # quad: contributions per slot


# ---------------------------------------------------------------- host prep
def _preprocess_edges(atom_index12, distances):
    """Sort contributions by destination atom, pack 4 same-destination
    contributions per slot, tile slots per 128-atom destination block
    padded to a uniform Tq tiles (same for all cores: one program)."""
    i0 = atom_index12[0].astype(np.int64)
    i1 = atom_index12[1].astype(np.int64)
    dest = np.concatenate([i0, i1])
    src = np.concatenate([i1, i0])
    dd = np.concatenate([distances, distances]).astype(np.float32)

    order = np.argsort(dest, kind="stable")
    dest_s = dest[order]
    src_s = src[order]
    dd_s = dd[order]

    d_a = np.bincount(dest_s, minlength=N)          # contributions per atom
    spa = (d_a + QM - 1) // QM                      # slots per atom
    spa2 = spa.reshape(NBLK, BLK)
    off_in_block = np.cumsum(spa2, axis=1) - spa2   # slot offset of atom
    slots_per_block = spa2.sum(axis=1)
    Tq = int(np.ceil(slots_per_block.max() / 128))
    KQ = NBLK_C * Tq

    cumd = np.cumsum(d_a)
    first_idx = (cumd - d_a)[dest_s]
    r = np.arange(dest_s.size) - first_idx          # rank within atom
    slot_in_atom = r // QM
    member = r % QM
    blk = dest_s // BLK
    aib = dest_s % BLK
    sib = off_in_block[blk, aib] + slot_in_atom     # slot within block
    t = sib // 128
    p = sib % 128
    core = dest_s // AC
    bb = blk - core * NBLK_C
    kq = bb * Tq + t                                # slot-tile column

    # dma_gather index list per block: linear index i = (t*4+m)*128 + p,
    # wrapped on 16 partitions: idx16[i%16, i//16], replicated 8x for the
    # Q7 cores. Column space per slot-tile: 4*128/16 = 32.
    i_in_block = (t * QM + member) * 128 + p
    eidx16 = np.zeros((NCORE, 16, KQ * 32), np.int16)
    dval = np.full((NCORE, 128, QM * KQ), 10.0, np.float32)  # plane layout
    dloc = np.zeros((NCORE, 128, KQ), np.float32)
    eidx16[core, i_in_block % 16, bb * (Tq * 32) + i_in_block // 16] = src_s
    dval[core, p, member * KQ + kq] = dd_s
    dloc[core, p, kq] = aib
    eidx = np.tile(eidx16, (1, 8, 1))
    return Tq, KQ, eidx, dval, dloc


# ---------------------------------------------------------------- device kernel
_CACHE = {}


def _build(Tq, dp2, df2):
    import concourse.bass as bass
    import concourse.bacc as bacc
    import concourse.mybir as mybir
    import concourse.tile as tile
    from concourse.masks import make_identity

    f32 = mybir.dt.float32
    bf16 = mybir.dt.bfloat16
    i32 = mybir.dt.int32
    i16 = mybir.dt.int16
    AF = mybir.ActivationFunctionType
    OP = mybir.AluOpType
    KQ = NBLK_C * Tq
    GQ = 2                    # slot-tiles per gather (1024-descriptor ring cap)
    NGB = (Tq + GQ - 1) // GQ  # gathers per block

    nc = bacc.Bacc("TRN2", target_bir_lowering=False, num_devices=NCORE,
                   num_swdge_queues=4)

    x_in = nc.dram_tensor("x", [AC, D], bf16, kind="ExternalInput")
    sp_in = nc.dram_tensor("species", [1, AC], i32, kind="ExternalInput")
    tc_in = nc.dram_tensor("tcharge", [1, MOL], f32, kind="ExternalInput")
    w1_in = nc.dram_tensor("W1", [S, D, H], bf16, kind="ExternalInput")
    w2_in = nc.dram_tensor("W2", [S, H, F], bf16, kind="ExternalInput")
    wn_in = nc.dram_tensor("Wn", [S, F, F], bf16, kind="ExternalInput")
    wf_in = nc.dram_tensor("Wf", [S, 2 * F, 1], bf16, kind="ExternalInput")
    eidx_in = nc.dram_tensor("eidx", [128, KQ * 32], i16, kind="ExternalInput")
    dval_in = nc.dram_tensor("dval", [128, QM * KQ], f32, kind="ExternalInput")
    dloc_in = nc.dram_tensor("dloc", [128, KQ], f32, kind="ExternalInput")
    out_t = nc.dram_tensor("out", [2, AC], f32, kind="ExternalOutput")

    nbr_local = nc.dram_tensor("nbr_local", [AC, 128], bf16)
    nbr_full_sh = nc.dram_tensor("nbr_full_sh", [N, 128], bf16,
                                 addr_space="Shared")
    nbr_full = nc.dram_tensor("nbr_full", [N, 128], bf16)

    with tile.TileContext(nc) as tc:
        # ---------------- persistent tiles
        _keep = []

        def _single(shape, dtype, name):
            t, free = tc.tile(shape, dtype, name=name)
            _keep.append(free)
            return t

        identb = _single([128, 128], bf16, "identb")
        make_identity(nc, identb[:, :])
        iotab = _single([128, Tq * 128], bf16, "iotab")
        internalT = _single([F, AC], bf16, "internalT")
        mergedT = _single([F, AC], bf16, "mergedT")
        nbrT = _single([F, AC], bf16, "nbrT")
        eqs = _single([S, AC], bf16, "eqs")
        spf = _single([1, AC], bf16, "spf")
        ones128 = _single([1, 128], bf16, "ones128")
        nc.vector.memset(ones128[:, :], 1.0)
        ones4c = _single([S, 1], bf16, "ones4c")
        nc.vector.memset(ones4c[:, :], 1.0)
        ones4r = _single([1, S], bf16, "ones4r")
        nc.vector.memset(ones4r[:, :], 1.0)
        xT = [_single([128, AC], bf16, f"xT{k}") for k in range(3)]
        idxs = _single([128, KQ * 32], i16, "idxs")
        wgtb = _single([128, QM * KQ], bf16, "wgtb")
        dlocb = _single([128, KQ], bf16, "dlocb")
        w1t = [[_single([128, H], bf16, f"w1_{s}_{k}") for k in range(3)]
               for s in range(S)]
        w2t = [[_single([96, F], bf16, f"w2_{s}_{k}") for k in range(2)]
               for s in range(S)]
        wnt = [_single([F, F], bf16, f"wn_{s}") for s in range(S)]
        wfi = _single([96, S], bf16, "wfi")
        wfm = _single([96, S], bf16, "wfm")
        tct = _single([1, MOL], f32, "tct")

        # ---------------- phase 0: init (loads + metadata, overlaps)
        with tc.tile_pool(name="init_pool", bufs=2) as ip, \
             tc.tile_pool(name="init_psum", bufs=2, space="PSUM") as ipp:
            iota_i = ip.tile([128, Tq * 128], i32, name="iota_i")
            nc.gpsimd.iota(iota_i[:, :], pattern=[[0, Tq], [1, 128]], base=0,
                           channel_multiplier=0)
            nc.vector.tensor_copy(iotab[:, :], iota_i[:, :])

            sp_i = ip.tile([1, AC], i32, name="sp_i")
            nc.sync.dma_start(sp_i[:, :], sp_in[:, :])
            nc.vector.tensor_copy(spf[:, :], sp_i[:, :])
            svec_i = ip.tile([S, 1], i32, name="svec_i")
            nc.gpsimd.iota(svec_i[:, :], pattern=[[0, 1]], base=0,
                           channel_multiplier=1)
            svec = ip.tile([S, 1], f32, name="svec")
            nc.vector.tensor_copy(svec[:, :], svec_i[:, :])
            for cch in range(NCH):
                sl = slice(cch * 512, (cch + 1) * 512)
                sp4 = ipp.tile([S, 512], f32, tag="sp4", name="sp4")
                nc.tensor.matmul(sp4[:, :], ones4r[:, :], spf[:, sl],
                                 start=True, stop=True)
                nc.vector.tensor_scalar(
                    out=eqs[:, sl], in0=sp4[:, :], scalar1=svec[:, :],
                    scalar2=None, op0=OP.is_equal)

            # edge metadata + decay weights
            nc.sync.dma_start(idxs[:, :], eidx_in[:, :])
            dvt = ip.tile([128, QM * KQ], f32, name="dvt")
            nc.sync.dma_start(dvt[:, :], dval_in[:, :])
            dlt = ip.tile([128, KQ], f32, name="dlt")
            nc.sync.dma_start(dlt[:, :], dloc_in[:, :])
            nc.vector.tensor_copy(dlocb[:, :], dlt[:, :])

            xx = ip.tile([128, QM * KQ], f32, name="xx")
            sc1 = ip.tile([128, QM * KQ], f32, name="sc1")
            wg = ip.tile([128, QM * KQ], f32, name="wg")
            nc.vector.tensor_scalar(out=xx[:, :], in0=dvt[:, :],
                                    scalar1=CUTOFF, scalar2=-1.0 / CUTOFF,
                                    op0=OP.subtract, op1=OP.mult)
            nc.vector.tensor_scalar(out=xx[:, :], in0=xx[:, :], scalar1=0.0,
                                    scalar2=1.0, op0=OP.max, op1=OP.min)
            nc.vector.tensor_scalar(out=sc1[:, :], in0=xx[:, :], scalar1=6.0,
                                    scalar2=-15.0, op0=OP.mult, op1=OP.add)
            nc.vector.tensor_tensor(out=sc1[:, :], in0=sc1[:, :],
                                    in1=xx[:, :], op=OP.mult)
            nc.vector.tensor_scalar(out=sc1[:, :], in0=sc1[:, :],
                                    scalar1=10.0, scalar2=None, op0=OP.add)
            nc.vector.tensor_tensor(out=wg[:, :], in0=xx[:, :], in1=xx[:, :],
                                    op=OP.mult)
            nc.vector.tensor_tensor(out=wg[:, :], in0=wg[:, :], in1=sc1[:, :],
                                    op=OP.mult)
            nc.vector.scalar_tensor_tensor(out=wg[:, :], in0=wg[:, :],
                                           scalar=float(dp2), in1=xx[:, :],
                                           op0=OP.mult, op1=OP.mult)
            nc.scalar.activation(sc1[:, :], dvt[:, :], AF.Exp,
                                 scale=-float(df2))
            nc.vector.tensor_tensor(out=wgtb[:, :], in0=wg[:, :],
                                    in1=sc1[:, :], op=OP.mult)

            # weights
            for s in range(S):
                for k in range(3):
                    nc.sync.dma_start(w1t[s][k][:, :],
                                      w1_in[s, k * 128:(k + 1) * 128, :])
                for k in range(2):
                    nc.sync.dma_start(w2t[s][k][:, :],
                                      w2_in[s, k * 96:(k + 1) * 96, :])
                nc.sync.dma_start(wnt[s][:, :], wn_in[s, :, :])
                nc.sync.dma_start(wfi[:, s:s + 1], wf_in[s, 0:96, 0:1])
                nc.sync.dma_start(wfm[:, s:s + 1], wf_in[s, 96:192, 0:1])
            nc.sync.dma_start(tct[:, :], tc_in[:, :])

            # x -> xT (transposed, bf16)
            with tc.tile_pool(name="tr_psum", bufs=4, space="PSUM") as trp, \
                 tc.tile_pool(name="tr_work", bufs=3) as twk:
                for t in range(NTA):
                    xa = twk.tile([128, D], bf16, tag="xa", name="xa")
                    nc.sync.dma_start(xa[:, :], x_in[t * 128:(t + 1) * 128, :])
                    for k in range(3):
                        tp = trp.tile([128, 128], bf16, tag="xtp", name="xtp")
                        nc.tensor.transpose(tp[:, :],
                                            xa[:, k * 128:(k + 1) * 128],
                                            identb[:, :])
                        nc.vector.tensor_copy(
                            xT[k][:, t * 128:(t + 1) * 128], tp[:, :])

        tc.strict_bb_all_engine_barrier()

        # ---------------- phase 1: per-species MLP (masked inputs,
        # PSUM-accumulated over species), staging nbr rows per chunk
        nbr_rows = nbr_local.rearrange("(t p) e -> p t e", p=128)
        with tc.tile_pool(name="mlp_work", bufs=3) as wk, \
             tc.tile_pool(name="mask_pool", bufs=S + 1) as mkp, \
             tc.tile_pool(name="mlp_psum", bufs=2, space="PSUM") as pp, \
             tc.tile_pool(name="mask_psum", bufs=1, space="PSUM") as mpp, \
             tc.tile_pool(name="acc_psum", bufs=1, space="PSUM") as app:
            for cch in range(NCH):
                sl = slice(cch * 512, (cch + 1) * 512)
                masks = []
                pv = app.tile([F, 512], f32, tag="pv", name="pv")
                for s in range(S):
                    eq_c = wk.tile([1, 512], bf16, tag="eq_c", name="eq_c")
                    nc.vector.tensor_scalar(
                        out=eq_c[:, :], in0=spf[:, sl], scalar1=float(s),
                        scalar2=None, op0=OP.is_equal)
                    mps = mpp.tile([128, 512], f32, tag="mps", name="mps")
                    nc.tensor.matmul(mps[:, :], ones128[:, :], eq_c[:, :],
                                     start=True, stop=True)
                    mask = mkp.tile([128, 512], bf16, tag=f"mask{s}",
                                    name=f"mask{s}")
                    nc.vector.tensor_copy(mask[:, :], mps[:, :])
                    masks.append(mask)

                    ph0 = pp.tile([96, 512], f32, tag="ph0", name="ph0")
                    ph1 = pp.tile([96, 512], f32, tag="ph1", name="ph1")
                    for k in range(3):
                        xm = wk.tile([128, 512], bf16, tag="xm", name="xm")
                        nc.vector.tensor_tensor(out=xm[:, :],
                                                in0=xT[k][:, sl],
                                                in1=mask[:, :], op=OP.mult)
                        nc.tensor.matmul(ph0[:, :], w1t[s][k][:, 0:96],
                                         xm[:, :], start=(k == 0),
                                         stop=(k == 2))
                        nc.tensor.matmul(ph1[:, :], w1t[s][k][:, 96:192],
                                         xm[:, :], start=(k == 0),
                                         stop=(k == 2))
                    h0 = wk.tile([96, 512], bf16, tag="h0", name="h0")
                    h1 = wk.tile([96, 512], bf16, tag="h1", name="h1")
                    nc.scalar.activation(h0[:, :], ph0[:, :],
                                         AF.Gelu_apprx_tanh)
                    nc.scalar.activation(h1[:, :], ph1[:, :],
                                         AF.Gelu_apprx_tanh)
                    nc.tensor.matmul(pv[:, :], w2t[s][0][:, :], h0[:, :],
                                     start=(s == 0), stop=False)
                    nc.tensor.matmul(pv[:, :], w2t[s][1][:, :], h1[:, :],
                                     start=False, stop=(s == S - 1))
                nc.vector.tensor_copy(internalT[:, sl], pv[:, :])

                pn = app.tile([F, 512], f32, tag="pn", name="pn")
                for s in range(S):
                    im = wk.tile([96, 512], bf16, tag="im", name="im")
                    nc.vector.tensor_tensor(out=im[:, :],
                                            in0=internalT[:, sl],
                                            in1=masks[s][0:96, :],
                                            op=OP.mult)
                    nc.tensor.matmul(pn[:, :], wnt[s][:, :], im[:, :],
                                     start=(s == 0), stop=(s == S - 1))
                nc.vector.tensor_copy(nbrT[:, sl], pn[:, :])

                # stage this chunk's nbr rows (transposed back, padded)
                for t in range(4):
                    tg = cch * 4 + t
                    ntp = mpp.tile([128, 96], bf16, tag="ntp", name="ntp")
                    nc.tensor.transpose(ntp[:, :],
                                        nbrT[:, tg * 128:(tg + 1) * 128],
                                        identb[0:96, 0:96])
                    strow = wk.tile([128, 128], bf16, tag="strow",
                                    name="strow")
                    nc.vector.memset(strow[:, 96:128], 0.0)
                    nc.vector.tensor_copy(strow[:, 0:96], ntp[:, :])
                    nc.sync.dma_start(nbr_rows[:, tg, :], strow[:, :])

        tc.strict_bb_all_engine_barrier()

        # ---------------- phase 2: AllGather nbr table + split bounce
        nc.gpsimd.collective_compute(
            "AllGather", mybir.AluOpType.bypass,
            replica_groups=[list(range(NCORE))],
            ins=[nbr_local[:, :]],
            outs=[nbr_full_sh[:, :]],
        )
        BCH = N // 16
        for q in range(16):
            nc.sync.dma_start(nbr_full[q * BCH:(q + 1) * BCH, :],
                              nbr_full_sh[q * BCH:(q + 1) * BCH, :])

        tc.strict_bb_all_engine_barrier()

        # ---------------- phase 3: edge aggregation (quad slots)
        with tc.tile_pool(name="msg_pool", bufs=2) as mgp, \
             tc.tile_pool(name="edge_work", bufs=2) as ew, \
             tc.tile_pool(name="edge_psum", bufs=2, space="PSUM") as epp:
            for b in range(NBLK_C):
                kqb = b * Tq
                # gathers in ring-sized pieces into one per-block buffer
                msg = mgp.tile([128, Tq * QM * 128], bf16, tag="msg",
                               name="msg")
                for g0 in range(0, Tq, GQ):
                    gn = min(GQ, Tq - g0)
                    nidx = gn * QM * 128
                    nc.gpsimd.dma_gather(
                        out_ap=msg[:, g0 * QM * 128:
                                   (g0 + gn) * QM * 128].rearrange(
                            "p (c e) -> p c e", e=128),
                        in_ap=nbr_full[:, :],
                        idxs_ap=idxs[:, (kqb + g0) * 32:(kqb + g0 + gn) * 32],
                        num_idxs=nidx,
                        num_idxs_reg=nidx,
                        elem_size=128)
                mv = msg[:, :].rearrange("p (g m f) -> p g m f", m=QM, f=128)
                # msgw = sum_m member_m * w_m   (weighted quad combine)
                acc = ew.tile([128, Tq * 96], bf16, tag="acc", name="acc")
                av = acc[:, :].rearrange("p (g f) -> p g f", f=96)
                w0 = wgtb[:, 0 * KQ + kqb:0 * KQ + kqb + Tq]
                nc.vector.tensor_tensor(
                    out=av, in0=mv[:, :, 0, 0:96],
                    in1=w0.to_broadcast([128, Tq, 96]), op=OP.mult)
                for m in range(1, QM):
                    tmp = ew.tile([128, Tq * 96], bf16, tag="tmp", name="tmp")
                    tv = tmp[:, :].rearrange("p (g f) -> p g f", f=96)
                    wm = wgtb[:, m * KQ + kqb:m * KQ + kqb + Tq]
                    nc.vector.tensor_tensor(
                        out=tv, in0=mv[:, :, m, 0:96],
                        in1=wm.to_broadcast([128, Tq, 96]), op=OP.mult)
                    nc.vector.tensor_tensor(out=av, in0=av, in1=tv,
                                            op=OP.add)
                # one-hot destination tiles
                oh = ew.tile([128, Tq * 128], bf16, tag="oh", name="oh")
                ov = oh[:, :].rearrange("p (g f) -> p g f", f=128)
                dl = dlocb[:, kqb:kqb + Tq]
                nc.vector.tensor_tensor(
                    out=ov,
                    in0=iotab[:, :].rearrange("p (g f) -> p g f", f=128),
                    in1=dl.to_broadcast([128, Tq, 128]), op=OP.is_equal)
                pm = epp.tile([F, 128], f32, tag="pm", name="pm")
                for t in range(Tq):
                    nc.tensor.matmul(
                        pm[:, :], acc[:, t * 96:(t + 1) * 96],
                        oh[:, t * 128:(t + 1) * 128],
                        start=(t == 0), stop=(t == Tq - 1))
                nc.vector.tensor_copy(mergedT[:, b * 128:(b + 1) * 128],
                                      pm[:, :])

        tc.strict_bb_all_engine_barrier()

        # ---------------- phase 4: final head + charge redistribution
        with tc.tile_pool(name="head_sbuf", bufs=1) as hp, \
             tc.tile_pool(name="head_work", bufs=3) as hw, \
             tc.tile_pool(name="head_psum", bufs=2, space="PSUM") as hpp:
            pre = hp.tile([1, AC], f32, tag="pre", name="pre")
            chg = hp.tile([1, AC], f32, tag="chg", name="chg")

            for cch in range(NCH):
                sl = slice(cch * 512, (cch + 1) * 512)
                pp4 = hpp.tile([S, 512], f32, tag="pp4", name="pp4")
                nc.tensor.matmul(pp4[:, :], wfi[:, :], internalT[:, sl],
                                 start=True, stop=False)
                nc.tensor.matmul(pp4[:, :], wfm[:, :], mergedT[:, sl],
                                 start=False, stop=True)
                sel = hw.tile([S, 512], bf16, tag="sel", name="sel")
                nc.vector.tensor_tensor(out=sel[:, :], in0=pp4[:, :],
                                        in1=eqs[:, sl], op=OP.mult)
                pr1 = hpp.tile([1, 512], f32, tag="pr1", name="pr1")
                nc.tensor.matmul(pr1[:, :], ones4c[:, :], sel[:, :],
                                 start=True, stop=True)
                nc.vector.tensor_copy(pre[:, sl], pr1[:, :])

            # per-molecule redistribution
            msum = hp.tile([1, MOL], f32, tag="msum", name="msum")
            nc.vector.tensor_reduce(
                out=msum[:, :],
                in_=pre[:, :].rearrange("p (m a) -> p m a", a=A),
                axis=mybir.AxisListType.X, op=OP.add)
            adj = hp.tile([1, MOL], f32, tag="adj", name="adj")
            nc.vector.tensor_tensor(out=adj[:, :], in0=tct[:, :],
                                    in1=msum[:, :], op=OP.subtract)
            nc.vector.tensor_scalar(out=adj[:, :], in0=adj[:, :],
                                    scalar1=1.0 / A, scalar2=None,
                                    op0=OP.mult)
            nc.vector.tensor_tensor(
                out=chg[:, :].rearrange("p (m a) -> p m a", a=A),
                in0=pre[:, :].rearrange("p (m a) -> p m a", a=A),
                in1=adj[:, :].to_broadcast([1, MOL, A]),
                op=OP.add)

            nc.sync.dma_start(out_t[0:1, :], chg[:, :])
            nc.sync.dma_start(out_t[1:2, :], pre[:, :])

        for free in reversed(_keep):
            free()

    nc.compile()
    return nc


def _get_nc(Tq, dp2, df2):
    key = (Tq, round(float(dp2), 9), round(float(df2), 9))
    if key not in _CACHE:
        _CACHE[key] = _build(Tq, dp2, df2)
    return _CACHE[key]


# ---------------------------------------------------------------- entry point
def kernel(species, in_features, atom_index12, distances, total_charges,
           W1, W2, Wn, Wf, decay_prefactor, decay_factor, _trace=False):
    import ml_dtypes
    from concourse.bass_utils import run_bass_kernel_spmd

    bf = ml_dtypes.bfloat16
    species = np.asarray(species, np.int32)
    in_features = np.ascontiguousarray(np.asarray(in_features, np.float32))
    atom_index12 = np.asarray(atom_index12, np.int32)
    distances = np.asarray(distances, np.float32)
    total_charges = np.asarray(total_charges, np.float32)
    W1b = np.ascontiguousarray(np.asarray(W1, np.float32).astype(bf))
    W2b = np.ascontiguousarray(np.asarray(W2, np.float32).astype(bf))
    Wnb = np.ascontiguousarray(np.asarray(Wn, np.float32).astype(bf))
    Wfb = np.ascontiguousarray(np.asarray(Wf, np.float32).astype(bf))
    dp2 = float(np.asarray(decay_prefactor)) ** 2
    df2 = float(np.asarray(decay_factor)) ** 2

    Tq, KQ, eidx, dval, dloc = _preprocess_edges(atom_index12, distances)
    nc = _get_nc(Tq, dp2, df2)

    in_maps = []
    for c in range(NCORE):
        in_maps.append({
            "x": np.ascontiguousarray(
                in_features[c * MOL:(c + 1) * MOL].reshape(AC, D).astype(bf)),
            "species": np.ascontiguousarray(
                species[c * MOL:(c + 1) * MOL].reshape(1, AC)),
            "tcharge": np.ascontiguousarray(
                total_charges[c * MOL:(c + 1) * MOL].reshape(1, MOL)),
            "W1": W1b, "W2": W2b, "Wn": Wnb, "Wf": Wfb,
            "eidx": np.ascontiguousarray(eidx[c]),
            "dval": np.ascontiguousarray(dval[c]),
            "dloc": np.ascontiguousarray(dloc[c]),
        })

    res = run_bass_kernel_spmd(nc, in_maps, core_ids=list(range(NCORE)),
                               trace=_trace)
    charges = np.empty((B, A), np.float32)
    precharges = np.empty((B, A), np.float32)
    for c in range(NCORE):
        o = res.results[c]["out"]
        charges[c * MOL:(c + 1) * MOL] = o[0].reshape(MOL, A)
        precharges[c * MOL:(c + 1) * MOL] = o[1].reshape(MOL, A)
    if _trace:
        kernel._last_results = res
    return species.reshape(B, A), charges, precharges


# revision 15
# speedup vs baseline: 2.7845x; 1.9946x over previous
"""Trainium2 Bass kernel for nn_DipoleEnergyLean (gnn_message_passing).

Strategy (8 NeuronCores, SPMD, bf16 compute / f32 accumulate):
  - Atoms sharded by molecule: core c owns molecules [32c,32c+32) = atoms
    [3072c, 3072c+3072). Weights replicated.
  - Per-species MLP via masked INPUTS accumulated in PSUM across species
    (gelu(0)=0 makes input-masking exact), feature-major layout.
  - Edge aggregation: contributions sorted by destination atom on the
    host and packed into "quad slots" (4 same-destination contributions
    per slot). Per 128-slot tile: one one-hot matmul scatters 512
    contributions. Gathers are batched ~GQ tiles (up to ~4.6K rows) per
    indirect DMA to amortize the ~1us SWDGE fixed cost. The nbr table is
    bf16 (256B rows) to halve gather traffic.
  - AllGather the bf16 nbr table, bounce Shared->Local split across all
    DMA queues, then per destination block accumulate T quad-tiles in
    PSUM via msgw^T @ onehot matmuls writing mergedT directly.
  - Final per-species head + per-molecule charge redistribution on-chip.

The harness calls kernel(**inputs) with the full unsharded arrays; this
file shards on the host, runs the SPMD Bass kernel on cores 0-7 via
run_bass_kernel_spmd, and reassembles the full output.
"""

import numpy as np

# ---------------------------------------------------------------- sizes
B, A, D, H, F = 256, 96, 384, 192, 96
S = 4
N = B * A                 # 24576 atoms
E = N * 32                # 786432 edges
CUTOFF = 5.2
NCORE = 8
AC = N // NCORE           # 3072 atoms / core
MOL = B // NCORE          # 32 molecules / core
BLK = 128                 # destination block (atoms)
NBLK = N // BLK           # 192 global blocks
NBLK_C = AC // BLK        # 24 blocks / core
NTA = AC // 128           # 24 atom tiles / core
NCH = AC // 512           # 6 atom chunks of 512
QM = 4                    # quad: contributions per slot


# ---------------------------------------------------------------- host prep
def _preprocess_edges(atom_index12, distances):
    """Sort contributions by destination atom, pack 4 same-destination
    contributions per slot, tile slots per 128-atom destination block
    padded to a uniform Tq tiles (same for all cores: one program)."""
    i0 = atom_index12[0].astype(np.int64)
    i1 = atom_index12[1].astype(np.int64)
    dest = np.concatenate([i0, i1])
    src = np.concatenate([i1, i0])
    dd = np.concatenate([distances, distances]).astype(np.float32)

    order = np.argsort(dest, kind="stable")
    dest_s = dest[order]
    src_s = src[order]
    dd_s = dd[order]

    d_a = np.bincount(dest_s, minlength=N)          # contributions per atom
    spa = (d_a + QM - 1) // QM                      # slots per atom
    spa2 = spa.reshape(NBLK, BLK)
    off_in_block = np.cumsum(spa2, axis=1) - spa2   # slot offset of atom
    slots_per_block = spa2.sum(axis=1)
    Tq = int(np.ceil(slots_per_block.max() / 128))
    KQ = NBLK_C * Tq

    cumd = np.cumsum(d_a)
    first_idx = (cumd - d_a)[dest_s]
    r = np.arange(dest_s.size) - first_idx          # rank within atom
    slot_in_atom = r // QM
    member = r % QM
    blk = dest_s // BLK
    aib = dest_s % BLK
    sib = off_in_block[blk, aib] + slot_in_atom     # slot within block
    t = sib // 128
    p = sib % 128
    core = dest_s // AC
    bb = blk - core * NBLK_C
    kq = bb * Tq + t                                # slot-tile column

    # dma_gather index list per block: linear index i = (t*4+m)*128 + p,
    # wrapped on 16 partitions: idx16[i%16, i//16], replicated 8x for the
    # Q7 cores. Column space per slot-tile: 4*128/16 = 32.
    i_in_block = (t * QM + member) * 128 + p
    eidx16 = np.zeros((NCORE, 16, KQ * 32), np.int16)
    dval = np.full((NCORE, 128, QM * KQ), 10.0, np.float32)  # plane layout
    dloc = np.zeros((NCORE, 128, KQ), np.float32)
    eidx16[core, i_in_block % 16, bb * (Tq * 32) + i_in_block // 16] = src_s
    dval[core, p, member * KQ + kq] = dd_s
    dloc[core, p, kq] = aib
    eidx = np.tile(eidx16, (1, 8, 1))
    return Tq, KQ, eidx, dval, dloc


# ---------------------------------------------------------------- device kernel
_CACHE = {}


def _build(Tq, dp2, df2):
    import concourse.bass as bass
    import concourse.bacc as bacc
    import concourse.mybir as mybir
    import concourse.tile as tile
    from concourse.masks import make_identity

    f32 = mybir.dt.float32
    bf16 = mybir.dt.bfloat16
    i32 = mybir.dt.int32
    i16 = mybir.dt.int16
    AF = mybir.ActivationFunctionType
    OP = mybir.AluOpType
    KQ = NBLK_C * Tq
    GQ = 2                    # slot-tiles per gather (1024-descriptor ring cap)
    NGB = (Tq + GQ - 1) // GQ  # gathers per block

    nc = bacc.Bacc("TRN2", target_bir_lowering=False, num_devices=NCORE,
                   num_swdge_queues=4)

    x_in = nc.dram_tensor("x", [AC, D], bf16, kind="ExternalInput")
    sp_in = nc.dram_tensor("species", [1, AC], i32, kind="ExternalInput")
    tc_in = nc.dram_tensor("tcharge", [1, MOL], f32, kind="ExternalInput")
    w1_in = nc.dram_tensor("W1", [S, D, H], bf16, kind="ExternalInput")
    w2_in = nc.dram_tensor("W2", [S, H, F], bf16, kind="ExternalInput")
    wn_in = nc.dram_tensor("Wn", [S, F, F], bf16, kind="ExternalInput")
    wf_in = nc.dram_tensor("Wf", [S, 2 * F, 1], bf16, kind="ExternalInput")
    eidx_in = nc.dram_tensor("eidx", [128, KQ * 32], i16, kind="ExternalInput")
    dval_in = nc.dram_tensor("dval", [128, QM * KQ], f32, kind="ExternalInput")
    dloc_in = nc.dram_tensor("dloc", [128, KQ], f32, kind="ExternalInput")
    out_t = nc.dram_tensor("out", [2, AC], f32, kind="ExternalOutput")

    nbr_local = nc.dram_tensor("nbr_local", [AC, 128], bf16)
    nbr_full_sh = nc.dram_tensor("nbr_full_sh", [N, 128], bf16,
                                 addr_space="Shared")
    nbr_full = nc.dram_tensor("nbr_full", [N, 128], bf16)

    with tile.TileContext(nc) as tc:
        # ---------------- persistent tiles
        _keep = []

        def _single(shape, dtype, name):
            t, free = tc.tile(shape, dtype, name=name)
            _keep.append(free)
            return t

        identb = _single([128, 128], bf16, "identb")
        make_identity(nc, identb[:, :])
        iotab = _single([128, Tq * 128], bf16, "iotab")
        internalT = _single([F, AC], bf16, "internalT")
        mergedT = _single([F, AC], bf16, "mergedT")
        nbrT = _single([F, AC], bf16, "nbrT")
        eqs = _single([S, AC], bf16, "eqs")
        spf = _single([1, AC], bf16, "spf")
        ones128 = _single([1, 128], bf16, "ones128")
        nc.vector.memset(ones128[:, :], 1.0)
        ones4c = _single([S, 1], bf16, "ones4c")
        nc.vector.memset(ones4c[:, :], 1.0)
        ones4r = _single([1, S], bf16, "ones4r")
        nc.vector.memset(ones4r[:, :], 1.0)
        xT = [_single([128, AC], bf16, f"xT{k}") for k in range(3)]
        idxs = _single([128, KQ * 32], i16, "idxs")
        wgtb = _single([128, QM * KQ], bf16, "wgtb")
        dlocb = _single([128, KQ], bf16, "dlocb")
        w1t = [[_single([128, H], bf16, f"w1_{s}_{k}") for k in range(3)]
               for s in range(S)]
        w2t = [[_single([96, F], bf16, f"w2_{s}_{k}") for k in range(2)]
               for s in range(S)]
        wnt = [_single([F, F], bf16, f"wn_{s}") for s in range(S)]
        wfi = _single([96, S], bf16, "wfi")
        wfm = _single([96, S], bf16, "wfm")
        tct = _single([1, MOL], f32, "tct")

        # ---------------- phase 0: init (loads + metadata, overlaps)
        with tc.tile_pool(name="init_pool", bufs=2) as ip, \
             tc.tile_pool(name="init_psum", bufs=2, space="PSUM") as ipp:
            iota_i = ip.tile([128, Tq * 128], i32, name="iota_i")
            nc.gpsimd.iota(iota_i[:, :], pattern=[[0, Tq], [1, 128]], base=0,
                           channel_multiplier=0)
            nc.vector.tensor_copy(iotab[:, :], iota_i[:, :])

            sp_i = ip.tile([1, AC], i32, name="sp_i")
            nc.sync.dma_start(sp_i[:, :], sp_in[:, :])
            nc.vector.tensor_copy(spf[:, :], sp_i[:, :])
            svec_i = ip.tile([S, 1], i32, name="svec_i")
            nc.gpsimd.iota(svec_i[:, :], pattern=[[0, 1]], base=0,
                           channel_multiplier=1)
            svec = ip.tile([S, 1], f32, name="svec")
            nc.vector.tensor_copy(svec[:, :], svec_i[:, :])
            for cch in range(NCH):
                sl = slice(cch * 512, (cch + 1) * 512)
                sp4 = ipp.tile([S, 512], f32, tag="sp4", name="sp4")
                nc.tensor.matmul(sp4[:, :], ones4r[:, :], spf[:, sl],
                                 start=True, stop=True)
                nc.vector.tensor_scalar(
                    out=eqs[:, sl], in0=sp4[:, :], scalar1=svec[:, :],
                    scalar2=None, op0=OP.is_equal)

            # edge metadata + decay weights
            nc.sync.dma_start(idxs[:, :], eidx_in[:, :])
            dvt = ip.tile([128, QM * KQ], f32, name="dvt")
            nc.sync.dma_start(dvt[:, :], dval_in[:, :])
            dlt = ip.tile([128, KQ], f32, name="dlt")
            nc.sync.dma_start(dlt[:, :], dloc_in[:, :])
            nc.vector.tensor_copy(dlocb[:, :], dlt[:, :])

            xx = ip.tile([128, QM * KQ], f32, name="xx")
            sc1 = ip.tile([128, QM * KQ], f32, name="sc1")
            wg = ip.tile([128, QM * KQ], f32, name="wg")
            nc.vector.tensor_scalar(out=xx[:, :], in0=dvt[:, :],
                                    scalar1=CUTOFF, scalar2=-1.0 / CUTOFF,
                                    op0=OP.subtract, op1=OP.mult)
            nc.vector.tensor_scalar(out=xx[:, :], in0=xx[:, :], scalar1=0.0,
                                    scalar2=1.0, op0=OP.max, op1=OP.min)
            nc.vector.tensor_scalar(out=sc1[:, :], in0=xx[:, :], scalar1=6.0,
                                    scalar2=-15.0, op0=OP.mult, op1=OP.add)
            nc.vector.tensor_tensor(out=sc1[:, :], in0=sc1[:, :],
                                    in1=xx[:, :], op=OP.mult)
            nc.vector.tensor_scalar(out=sc1[:, :], in0=sc1[:, :],
                                    scalar1=10.0, scalar2=None, op0=OP.add)
            nc.vector.tensor_tensor(out=wg[:, :], in0=xx[:, :], in1=xx[:, :],
                                    op=OP.mult)
            nc.vector.tensor_tensor(out=wg[:, :], in0=wg[:, :], in1=sc1[:, :],
                                    op=OP.mult)
            nc.vector.scalar_tensor_tensor(out=wg[:, :], in0=wg[:, :],
                                           scalar=float(dp2), in1=xx[:, :],
                                           op0=OP.mult, op1=OP.mult)
            nc.scalar.activation(sc1[:, :], dvt[:, :], AF.Exp,
                                 scale=-float(df2))
            nc.vector.tensor_tensor(out=wgtb[:, :], in0=wg[:, :],
                                    in1=sc1[:, :], op=OP.mult)

            # weights
            for s in range(S):
                for k in range(3):
                    nc.sync.dma_start(w1t[s][k][:, :],
                                      w1_in[s, k * 128:(k + 1) * 128, :])
                for k in range(2):
                    nc.sync.dma_start(w2t[s][k][:, :],
                                      w2_in[s, k * 96:(k + 1) * 96, :])
                nc.sync.dma_start(wnt[s][:, :], wn_in[s, :, :])
                nc.sync.dma_start(wfi[:, s:s + 1], wf_in[s, 0:96, 0:1])
                nc.sync.dma_start(wfm[:, s:s + 1], wf_in[s, 96:192, 0:1])
            nc.sync.dma_start(tct[:, :], tc_in[:, :])

            # x -> xT (transposed, bf16)
            with tc.tile_pool(name="tr_psum", bufs=4, space="PSUM") as trp, \
                 tc.tile_pool(name="tr_work", bufs=3) as twk:
                for t in range(NTA):
                    xa = twk.tile([128, D], bf16, tag="xa", name="xa")
                    nc.sync.dma_start(xa[:, :], x_in[t * 128:(t + 1) * 128, :])
                    for k in range(3):
                        tp = trp.tile([128, 128], bf16, tag="xtp", name="xtp")
                        nc.tensor.transpose(tp[:, :],
                                            xa[:, k * 128:(k + 1) * 128],
                                            identb[:, :])
                        nc.vector.tensor_copy(
                            xT[k][:, t * 128:(t + 1) * 128], tp[:, :])

        tc.strict_bb_all_engine_barrier()

        # ---------------- phase 1: per-species MLP (masked inputs,
        # PSUM-accumulated over species), staging nbr rows per chunk
        nbr_rows = nbr_local.rearrange("(t p) e -> p t e", p=128)
        with tc.tile_pool(name="mlp_work", bufs=3) as wk, \
             tc.tile_pool(name="mask_pool", bufs=S + 1) as mkp, \
             tc.tile_pool(name="mlp_psum", bufs=2, space="PSUM") as pp, \
             tc.tile_pool(name="mask_psum", bufs=1, space="PSUM") as mpp, \
             tc.tile_pool(name="acc_psum", bufs=1, space="PSUM") as app:
            for cch in range(NCH):
                sl = slice(cch * 512, (cch + 1) * 512)
                masks = []
                pv = app.tile([F, 512], f32, tag="pv", name="pv")
                for s in range(S):
                    eq_c = wk.tile([1, 512], bf16, tag="eq_c", name="eq_c")
                    nc.vector.tensor_scalar(
                        out=eq_c[:, :], in0=spf[:, sl], scalar1=float(s),
                        scalar2=None, op0=OP.is_equal)
                    mps = mpp.tile([128, 512], f32, tag="mps", name="mps")
                    nc.tensor.matmul(mps[:, :], ones128[:, :], eq_c[:, :],
                                     start=True, stop=True)
                    mask = mkp.tile([128, 512], bf16, tag=f"mask{s}",
                                    name=f"mask{s}")
                    nc.vector.tensor_copy(mask[:, :], mps[:, :])
                    masks.append(mask)

                    ph0 = pp.tile([96, 512], f32, tag="ph0", name="ph0")
                    ph1 = pp.tile([96, 512], f32, tag="ph1", name="ph1")
                    for k in range(3):
                        xm = wk.tile([128, 512], bf16, tag="xm", name="xm")
                        nc.vector.tensor_tensor(out=xm[:, :],
                                                in0=xT[k][:, sl],
                                                in1=mask[:, :], op=OP.mult)
                        nc.tensor.matmul(ph0[:, :], w1t[s][k][:, 0:96],
                                         xm[:, :], start=(k == 0),
                                         stop=(k == 2))
                        nc.tensor.matmul(ph1[:, :], w1t[s][k][:, 96:192],
                                         xm[:, :], start=(k == 0),
                                         stop=(k == 2))
                    h0 = wk.tile([96, 512], bf16, tag="h0", name="h0")
                    h1 = wk.tile([96, 512], bf16, tag="h1", name="h1")
                    nc.scalar.activation(h0[:, :], ph0[:, :],
                                         AF.Gelu_apprx_tanh)
                    nc.scalar.activation(h1[:, :], ph1[:, :],
                                         AF.Gelu_apprx_tanh)
                    nc.tensor.matmul(pv[:, :], w2t[s][0][:, :], h0[:, :],
                                     start=(s == 0), stop=False)
                    nc.tensor.matmul(pv[:, :], w2t[s][1][:, :], h1[:, :],
                                     start=False, stop=(s == S - 1))
                nc.vector.tensor_copy(internalT[:, sl], pv[:, :])

                pn = app.tile([F, 512], f32, tag="pn", name="pn")
                for s in range(S):
                    im = wk.tile([96, 512], bf16, tag="im", name="im")
                    nc.vector.tensor_tensor(out=im[:, :],
                                            in0=internalT[:, sl],
                                            in1=masks[s][0:96, :],
                                            op=OP.mult)
                    nc.tensor.matmul(pn[:, :], wnt[s][:, :], im[:, :],
                                     start=(s == 0), stop=(s == S - 1))
                nc.vector.tensor_copy(nbrT[:, sl], pn[:, :])

                # stage this chunk's nbr rows (transposed back, padded)
                for t in range(4):
                    tg = cch * 4 + t
                    ntp = mpp.tile([128, 96], bf16, tag="ntp", name="ntp")
                    nc.tensor.transpose(ntp[:, :],
                                        nbrT[:, tg * 128:(tg + 1) * 128],
                                        identb[0:96, 0:96])
                    strow = wk.tile([128, 128], bf16, tag="strow",
                                    name="strow")
                    nc.vector.memset(strow[:, 96:128], 0.0)
                    nc.vector.tensor_copy(strow[:, 0:96], ntp[:, :])
                    nc.sync.dma_start(nbr_rows[:, tg, :], strow[:, :])

        tc.strict_bb_all_engine_barrier()

        # ---------------- phase 2: AllGather nbr table + split bounce
        nc.gpsimd.collective_compute(
            "AllGather", mybir.AluOpType.bypass,
            replica_groups=[list(range(NCORE))],
            ins=[nbr_local[:, :]],
            outs=[nbr_full_sh[:, :]],
        )
        BCH = N // 16
        for q in range(16):
            nc.sync.dma_start(nbr_full[q * BCH:(q + 1) * BCH, :],
                              nbr_full_sh[q * BCH:(q + 1) * BCH, :])

        tc.strict_bb_all_engine_barrier()

        # ---------------- phase 3: edge aggregation (quad slots)
        with tc.tile_pool(name="msg_pool", bufs=2) as mgp, \
             tc.tile_pool(name="edge_work", bufs=2) as ew, \
             tc.tile_pool(name="edge_psum", bufs=2, space="PSUM") as epp:
            for b in range(NBLK_C):
                kqb = b * Tq
                # gathers in ring-sized pieces into one per-block buffer
                msg = mgp.tile([128, Tq * QM * 128], bf16, tag="msg",
                               name="msg")
                for g0 in range(0, Tq, GQ):
                    gn = min(GQ, Tq - g0)
                    nidx = gn * QM * 128
                    nc.gpsimd.dma_gather(
                        out_ap=msg[:, g0 * QM * 128:
                                   (g0 + gn) * QM * 128].rearrange(
                            "p (c e) -> p c e", e=128),
                        in_ap=nbr_full[:, :],
                        idxs_ap=idxs[:, (kqb + g0) * 32:(kqb + g0 + gn) * 32],
                        num_idxs=nidx,
                        num_idxs_reg=nidx,
                        elem_size=128,
                        queue_num=(b * NGB + g0 // GQ) % 4)
                mv = msg[:, :].rearrange("p (g m f) -> p g m f", m=QM, f=128)
                # msgw = sum_m member_m * w_m   (weighted quad combine)
                acc = ew.tile([128, Tq * 96], bf16, tag="acc", name="acc")
                av = acc[:, :].rearrange("p (g f) -> p g f", f=96)
                w0 = wgtb[:, 0 * KQ + kqb:0 * KQ + kqb + Tq]
                nc.vector.tensor_tensor(
                    out=av, in0=mv[:, :, 0, 0:96],
                    in1=w0.to_broadcast([128, Tq, 96]), op=OP.mult)
                for m in range(1, QM):
                    tmp = ew.tile([128, Tq * 96], bf16, tag="tmp", name="tmp")
                    tv = tmp[:, :].rearrange("p (g f) -> p g f", f=96)
                    wm = wgtb[:, m * KQ + kqb:m * KQ + kqb + Tq]
                    nc.vector.tensor_tensor(
                        out=tv, in0=mv[:, :, m, 0:96],
                        in1=wm.to_broadcast([128, Tq, 96]), op=OP.mult)
                    nc.vector.tensor_tensor(out=av, in0=av, in1=tv,
                                            op=OP.add)
                # one-hot destination tiles
                oh = ew.tile([128, Tq * 128], bf16, tag="oh", name="oh")
                ov = oh[:, :].rearrange("p (g f) -> p g f", f=128)
                dl = dlocb[:, kqb:kqb + Tq]
                nc.vector.tensor_tensor(
                    out=ov,
                    in0=iotab[:, :].rearrange("p (g f) -> p g f", f=128),
                    in1=dl.to_broadcast([128, Tq, 128]), op=OP.is_equal)
                pm = epp.tile([F, 128], f32, tag="pm", name="pm")
                for t in range(Tq):
                    nc.tensor.matmul(
                        pm[:, :], acc[:, t * 96:(t + 1) * 96],
                        oh[:, t * 128:(t + 1) * 128],
                        start=(t == 0), stop=(t == Tq - 1))
                nc.vector.tensor_copy(mergedT[:, b * 128:(b + 1) * 128],
                                      pm[:, :])

        tc.strict_bb_all_engine_barrier()

        # ---------------- phase 4: final head + charge redistribution
        with tc.tile_pool(name="head_sbuf", bufs=1) as hp, \
             tc.tile_pool(name="head_work", bufs=3) as hw, \
             tc.tile_pool(name="head_psum", bufs=2, space="PSUM") as hpp:
            pre = hp.tile([1, AC], f32, tag="pre", name="pre")
            chg = hp.tile([1, AC], f32, tag="chg", name="chg")

            for cch in range(NCH):
                sl = slice(cch * 512, (cch + 1) * 512)
                pp4 = hpp.tile([S, 512], f32, tag="pp4", name="pp4")
                nc.tensor.matmul(pp4[:, :], wfi[:, :], internalT[:, sl],
                                 start=True, stop=False)
                nc.tensor.matmul(pp4[:, :], wfm[:, :], mergedT[:, sl],
                                 start=False, stop=True)
                sel = hw.tile([S, 512], bf16, tag="sel", name="sel")
                nc.vector.tensor_tensor(out=sel[:, :], in0=pp4[:, :],
                                        in1=eqs[:, sl], op=OP.mult)
                pr1 = hpp.tile([1, 512], f32, tag="pr1", name="pr1")
                nc.tensor.matmul(pr1[:, :], ones4c[:, :], sel[:, :],
                                 start=True, stop=True)
                nc.vector.tensor_copy(pre[:, sl], pr1[:, :])

            # per-molecule redistribution
            msum = hp.tile([1, MOL], f32, tag="msum", name="msum")
            nc.vector.tensor_reduce(
                out=msum[:, :],
                in_=pre[:, :].rearrange("p (m a) -> p m a", a=A),
                axis=mybir.AxisListType.X, op=OP.add)
            adj = hp.tile([1, MOL], f32, tag="adj", name="adj")
            nc.vector.tensor_tensor(out=adj[:, :], in0=tct[:, :],
                                    in1=msum[:, :], op=OP.subtract)
            nc.vector.tensor_scalar(out=adj[:, :], in0=adj[:, :],
                                    scalar1=1.0 / A, scalar2=None,
                                    op0=OP.mult)
            nc.vector.tensor_tensor(
                out=chg[:, :].rearrange("p (m a) -> p m a", a=A),
                in0=pre[:, :].rearrange("p (m a) -> p m a", a=A),
                in1=adj[:, :].to_broadcast([1, MOL, A]),
                op=OP.add)

            nc.sync.dma_start(out_t[0:1, :], chg[:, :])
            nc.sync.dma_start(out_t[1:2, :], pre[:, :])

        for free in reversed(_keep):
            free()

    nc.compile()
    return nc


def _get_nc(Tq, dp2, df2):
    key = (Tq, round(float(dp2), 9), round(float(df2), 9))
    if key not in _CACHE:
        _CACHE[key] = _build(Tq, dp2, df2)
    return _CACHE[key]


# ---------------------------------------------------------------- entry point
def kernel(species, in_features, atom_index12, distances, total_charges,
           W1, W2, Wn, Wf, decay_prefactor, decay_factor, _trace=False):
    import ml_dtypes
    from concourse.bass_utils import run_bass_kernel_spmd

    bf = ml_dtypes.bfloat16
    species = np.asarray(species, np.int32)
    in_features = np.ascontiguousarray(np.asarray(in_features, np.float32))
    atom_index12 = np.asarray(atom_index12, np.int32)
    distances = np.asarray(distances, np.float32)
    total_charges = np.asarray(total_charges, np.float32)
    W1b = np.ascontiguousarray(np.asarray(W1, np.float32).astype(bf))
    W2b = np.ascontiguousarray(np.asarray(W2, np.float32).astype(bf))
    Wnb = np.ascontiguousarray(np.asarray(Wn, np.float32).astype(bf))
    Wfb = np.ascontiguousarray(np.asarray(Wf, np.float32).astype(bf))
    dp2 = float(np.asarray(decay_prefactor)) ** 2
    df2 = float(np.asarray(decay_factor)) ** 2

    Tq, KQ, eidx, dval, dloc = _preprocess_edges(atom_index12, distances)
    nc = _get_nc(Tq, dp2, df2)

    in_maps = []
    for c in range(NCORE):
        in_maps.append({
            "x": np.ascontiguousarray(
                in_features[c * MOL:(c + 1) * MOL].reshape(AC, D).astype(bf)),
            "species": np.ascontiguousarray(
                species[c * MOL:(c + 1) * MOL].reshape(1, AC)),
            "tcharge": np.ascontiguousarray(
                total_charges[c * MOL:(c + 1) * MOL].reshape(1, MOL)),
            "W1": W1b, "W2": W2b, "Wn": Wnb, "Wf": Wfb,
            "eidx": np.ascontiguousarray(eidx[c]),
            "dval": np.ascontiguousarray(dval[c]),
            "dloc": np.ascontiguousarray(dloc[c]),
        })

    res = run_bass_kernel_spmd(nc, in_maps, core_ids=list(range(NCORE)),
                               trace=_trace)
    charges = np.empty((B, A), np.float32)
    precharges = np.empty((B, A), np.float32)
    for c in range(NCORE):
        o = res.results[c]["out"]
        charges[c * MOL:(c + 1) * MOL] = o[0].reshape(MOL, A)
        precharges[c * MOL:(c + 1) * MOL] = o[1].reshape(MOL, A)
    if _trace:
        kernel._last_results = res
    return species.reshape(B, A), charges, precharges


# revision 17
# speedup vs baseline: 3.4183x; 1.2276x over previous
"""Trainium2 Bass kernel for nn_DipoleEnergyLean (gnn_message_passing).

Strategy (8 NeuronCores, SPMD, bf16 compute / f32 accumulate):
  - Atoms sharded by molecule: core c owns molecules [32c,32c+32) = atoms
    [3072c, 3072c+3072). Weights replicated.
  - Per-species MLP via masked INPUTS accumulated in PSUM across species
    (gelu(0)=0 makes input-masking exact), feature-major layout.
  - Edge aggregation: contributions sorted by destination atom on the
    host and packed into "quad slots" (4 same-destination contributions
    per slot). Per 128-slot tile: one one-hot matmul scatters 512
    contributions. Gathers are batched ~GQ tiles (up to ~4.6K rows) per
    indirect DMA to amortize the ~1us SWDGE fixed cost. The nbr table is
    bf16 (256B rows) to halve gather traffic.
  - AllGather the bf16 nbr table, bounce Shared->Local split across all
    DMA queues, then per destination block accumulate T quad-tiles in
    PSUM via msgw^T @ onehot matmuls writing mergedT directly.
  - Final per-species head + per-molecule charge redistribution on-chip.

The harness calls kernel(**inputs) with the full unsharded arrays; this
file shards on the host, runs the SPMD Bass kernel on cores 0-7 via
run_bass_kernel_spmd, and reassembles the full output.
"""

import numpy as np

# ---------------------------------------------------------------- sizes
B, A, D, H, F = 256, 96, 384, 192, 96
S = 4
N = B * A                 # 24576 atoms
E = N * 32                # 786432 edges
CUTOFF = 5.2
NCORE = 8
AC = N // NCORE           # 3072 atoms / core
MOL = B // NCORE          # 32 molecules / core
BLK = 128                 # destination block (atoms)
NBLK = N // BLK           # 192 global blocks
NBLK_C = AC // BLK        # 24 blocks / core
NTA = AC // 128           # 24 atom tiles / core
NCH = AC // 512           # 6 atom chunks of 512
QM = 4                    # quad: contributions per slot


# ---------------------------------------------------------------- host prep
def _preprocess_edges(atom_index12, distances):
    """Sort contributions by destination atom, pack 4 same-destination
    contributions per slot, tile slots per 128-atom destination block
    padded to a uniform Tq tiles (same for all cores: one program)."""
    i0 = atom_index12[0].astype(np.int64)
    i1 = atom_index12[1].astype(np.int64)
    dest = np.concatenate([i0, i1])
    src = np.concatenate([i1, i0])
    dd = np.concatenate([distances, distances]).astype(np.float32)

    order = np.argsort(dest, kind="stable")
    dest_s = dest[order]
    src_s = src[order]
    dd_s = dd[order]

    d_a = np.bincount(dest_s, minlength=N)          # contributions per atom
    spa = (d_a + QM - 1) // QM                      # slots per atom
    spa2 = spa.reshape(NBLK, BLK)
    off_in_block = np.cumsum(spa2, axis=1) - spa2   # slot offset of atom
    slots_per_block = spa2.sum(axis=1)
    Tq = int(np.ceil(slots_per_block.max() / 128))
    KQ = NBLK_C * Tq

    cumd = np.cumsum(d_a)
    first_idx = (cumd - d_a)[dest_s]
    r = np.arange(dest_s.size) - first_idx          # rank within atom
    slot_in_atom = r // QM
    member = r % QM
    blk = dest_s // BLK
    aib = dest_s % BLK
    sib = off_in_block[blk, aib] + slot_in_atom     # slot within block
    t = sib // 128
    p = sib % 128
    core = dest_s // AC
    bb = blk - core * NBLK_C
    kq = bb * Tq + t                                # slot-tile column

    # dma_gather index list per block: member-plane-major linear index
    # i = (m*Tq + t)*128 + p so each member plane is contiguous in SBUF,
    # wrapped on 16 partitions: idx16[i%16, i//16], replicated 8x for the
    # Q7 cores. Column space per slot-tile: 4*128/16 = 32.
    i_in_block = (member * Tq + t) * 128 + p
    eidx16 = np.zeros((NCORE, 16, KQ * 32), np.int16)
    dval = np.full((NCORE, 128, QM * KQ), 10.0, np.float32)  # plane layout
    dloc = np.zeros((NCORE, 128, KQ), np.float32)
    eidx16[core, i_in_block % 16, bb * (Tq * 32) + i_in_block // 16] = src_s
    dval[core, p, member * KQ + kq] = dd_s
    dloc[core, p, kq] = aib
    eidx = np.tile(eidx16, (1, 8, 1))
    return Tq, KQ, eidx, dval, dloc


# ---------------------------------------------------------------- device kernel
_CACHE = {}


def _build(Tq, dp2, df2):
    import concourse.bass as bass
    import concourse.bacc as bacc
    import concourse.mybir as mybir
    import concourse.tile as tile
    from concourse.masks import make_identity

    f32 = mybir.dt.float32
    bf16 = mybir.dt.bfloat16
    i32 = mybir.dt.int32
    i16 = mybir.dt.int16
    AF = mybir.ActivationFunctionType
    OP = mybir.AluOpType
    KQ = NBLK_C * Tq
    GQ = 2                    # slot-tiles per gather (1024-descriptor ring cap)
    NGB = (Tq + GQ - 1) // GQ  # gathers per block

    nc = bacc.Bacc("TRN2", target_bir_lowering=False, num_devices=NCORE,
                   num_swdge_queues=4)

    x_in = nc.dram_tensor("x", [AC, D], bf16, kind="ExternalInput")
    sp_in = nc.dram_tensor("species", [1, AC], i32, kind="ExternalInput")
    tc_in = nc.dram_tensor("tcharge", [1, MOL], f32, kind="ExternalInput")
    w1_in = nc.dram_tensor("W1", [S, D, H], bf16, kind="ExternalInput")
    w2_in = nc.dram_tensor("W2", [S, H, F], bf16, kind="ExternalInput")
    wn_in = nc.dram_tensor("Wn", [S, F, F], bf16, kind="ExternalInput")
    wf_in = nc.dram_tensor("Wf", [S, 2 * F, 1], bf16, kind="ExternalInput")
    eidx_in = nc.dram_tensor("eidx", [128, KQ * 32], i16, kind="ExternalInput")
    dval_in = nc.dram_tensor("dval", [128, QM * KQ], f32, kind="ExternalInput")
    dloc_in = nc.dram_tensor("dloc", [128, KQ], f32, kind="ExternalInput")
    out_t = nc.dram_tensor("out", [2, AC], f32, kind="ExternalOutput")

    nbr_local = nc.dram_tensor("nbr_local", [AC, 128], bf16)
    nbr_full_sh = nc.dram_tensor("nbr_full_sh", [N, 128], bf16,
                                 addr_space="Shared")
    nbr_full = nc.dram_tensor("nbr_full", [N, 128], bf16)

    with tile.TileContext(nc) as tc:
        # ---------------- persistent tiles
        _keep = []

        def _single(shape, dtype, name):
            t, free = tc.tile(shape, dtype, name=name)
            _keep.append(free)
            return t

        identb = _single([128, 128], bf16, "identb")
        make_identity(nc, identb[:, :])
        iotab = _single([128, Tq * 128], bf16, "iotab")
        internalT = _single([F, AC], bf16, "internalT")
        mergedT = _single([F, AC], bf16, "mergedT")
        nbrT = _single([F, AC], bf16, "nbrT")
        eqs = _single([S, AC], bf16, "eqs")
        spf = _single([1, AC], bf16, "spf")
        ones128 = _single([1, 128], bf16, "ones128")
        nc.vector.memset(ones128[:, :], 1.0)
        ones4c = _single([S, 1], bf16, "ones4c")
        nc.vector.memset(ones4c[:, :], 1.0)
        ones4r = _single([1, S], bf16, "ones4r")
        nc.vector.memset(ones4r[:, :], 1.0)
        xT = [_single([128, AC], bf16, f"xT{k}") for k in range(3)]
        idxs = _single([128, KQ * 32], i16, "idxs")
        wgtb = _single([128, QM * KQ], bf16, "wgtb")
        dlocb = _single([128, KQ], bf16, "dlocb")
        w1t = [[_single([128, H], bf16, f"w1_{s}_{k}") for k in range(3)]
               for s in range(S)]
        w2t = [[_single([96, F], bf16, f"w2_{s}_{k}") for k in range(2)]
               for s in range(S)]
        wnt = [_single([F, F], bf16, f"wn_{s}") for s in range(S)]
        wfi = _single([96, S], bf16, "wfi")
        wfm = _single([96, S], bf16, "wfm")
        tct = _single([1, MOL], f32, "tct")

        # ---------------- phase 0: init (loads + metadata, overlaps)
        with tc.tile_pool(name="init_pool", bufs=2) as ip, \
             tc.tile_pool(name="init_psum", bufs=2, space="PSUM") as ipp:
            iota_i = ip.tile([128, Tq * 128], i32, name="iota_i")
            nc.gpsimd.iota(iota_i[:, :], pattern=[[0, Tq], [1, 128]], base=0,
                           channel_multiplier=0)
            nc.vector.tensor_copy(iotab[:, :], iota_i[:, :])

            sp_i = ip.tile([1, AC], i32, name="sp_i")
            nc.sync.dma_start(sp_i[:, :], sp_in[:, :])
            nc.vector.tensor_copy(spf[:, :], sp_i[:, :])
            svec_i = ip.tile([S, 1], i32, name="svec_i")
            nc.gpsimd.iota(svec_i[:, :], pattern=[[0, 1]], base=0,
                           channel_multiplier=1)
            svec = ip.tile([S, 1], f32, name="svec")
            nc.vector.tensor_copy(svec[:, :], svec_i[:, :])
            for cch in range(NCH):
                sl = slice(cch * 512, (cch + 1) * 512)
                sp4 = ipp.tile([S, 512], f32, tag="sp4", name="sp4")
                nc.tensor.matmul(sp4[:, :], ones4r[:, :], spf[:, sl],
                                 start=True, stop=True)
                nc.vector.tensor_scalar(
                    out=eqs[:, sl], in0=sp4[:, :], scalar1=svec[:, :],
                    scalar2=None, op0=OP.is_equal)

            # edge metadata + decay weights
            nc.sync.dma_start(idxs[:, :], eidx_in[:, :])
            dvt = ip.tile([128, QM * KQ], f32, name="dvt")
            nc.sync.dma_start(dvt[:, :], dval_in[:, :])
            dlt = ip.tile([128, KQ], f32, name="dlt")
            nc.sync.dma_start(dlt[:, :], dloc_in[:, :])
            nc.vector.tensor_copy(dlocb[:, :], dlt[:, :])

            xx = ip.tile([128, QM * KQ], f32, name="xx")
            sc1 = ip.tile([128, QM * KQ], f32, name="sc1")
            wg = ip.tile([128, QM * KQ], f32, name="wg")
            nc.vector.tensor_scalar(out=xx[:, :], in0=dvt[:, :],
                                    scalar1=CUTOFF, scalar2=-1.0 / CUTOFF,
                                    op0=OP.subtract, op1=OP.mult)
            nc.vector.tensor_scalar(out=xx[:, :], in0=xx[:, :], scalar1=0.0,
                                    scalar2=1.0, op0=OP.max, op1=OP.min)
            nc.vector.tensor_scalar(out=sc1[:, :], in0=xx[:, :], scalar1=6.0,
                                    scalar2=-15.0, op0=OP.mult, op1=OP.add)
            nc.vector.tensor_tensor(out=sc1[:, :], in0=sc1[:, :],
                                    in1=xx[:, :], op=OP.mult)
            nc.vector.tensor_scalar(out=sc1[:, :], in0=sc1[:, :],
                                    scalar1=10.0, scalar2=None, op0=OP.add)
            nc.vector.tensor_tensor(out=wg[:, :], in0=xx[:, :], in1=xx[:, :],
                                    op=OP.mult)
            nc.vector.tensor_tensor(out=wg[:, :], in0=wg[:, :], in1=sc1[:, :],
                                    op=OP.mult)
            nc.vector.scalar_tensor_tensor(out=wg[:, :], in0=wg[:, :],
                                           scalar=float(dp2), in1=xx[:, :],
                                           op0=OP.mult, op1=OP.mult)
            nc.scalar.activation(sc1[:, :], dvt[:, :], AF.Exp,
                                 scale=-float(df2))
            nc.vector.tensor_tensor(out=wgtb[:, :], in0=wg[:, :],
                                    in1=sc1[:, :], op=OP.mult)

            # weights
            for s in range(S):
                for k in range(3):
                    nc.sync.dma_start(w1t[s][k][:, :],
                                      w1_in[s, k * 128:(k + 1) * 128, :])
                for k in range(2):
                    nc.sync.dma_start(w2t[s][k][:, :],
                                      w2_in[s, k * 96:(k + 1) * 96, :])
                nc.sync.dma_start(wnt[s][:, :], wn_in[s, :, :])
                nc.sync.dma_start(wfi[:, s:s + 1], wf_in[s, 0:96, 0:1])
                nc.sync.dma_start(wfm[:, s:s + 1], wf_in[s, 96:192, 0:1])
            nc.sync.dma_start(tct[:, :], tc_in[:, :])

            # x -> xT (transposed, bf16)
            with tc.tile_pool(name="tr_psum", bufs=4, space="PSUM") as trp, \
                 tc.tile_pool(name="tr_work", bufs=3) as twk:
                for t in range(NTA):
                    xa = twk.tile([128, D], bf16, tag="xa", name="xa")
                    nc.sync.dma_start(xa[:, :], x_in[t * 128:(t + 1) * 128, :])
                    for k in range(3):
                        tp = trp.tile([128, 128], bf16, tag="xtp", name="xtp")
                        nc.tensor.transpose(tp[:, :],
                                            xa[:, k * 128:(k + 1) * 128],
                                            identb[:, :])
                        nc.vector.tensor_copy(
                            xT[k][:, t * 128:(t + 1) * 128], tp[:, :])

        tc.strict_bb_all_engine_barrier()

        # ---------------- phase 1: per-species MLP (masked inputs,
        # PSUM-accumulated over species), staging nbr rows per chunk
        nbr_rows = nbr_local.rearrange("(t p) e -> p t e", p=128)
        with tc.tile_pool(name="mlp_work", bufs=3) as wk, \
             tc.tile_pool(name="mask_pool", bufs=S + 1) as mkp, \
             tc.tile_pool(name="mlp_psum", bufs=2, space="PSUM") as pp, \
             tc.tile_pool(name="mask_psum", bufs=1, space="PSUM") as mpp, \
             tc.tile_pool(name="acc_psum", bufs=1, space="PSUM") as app:
            for cch in range(NCH):
                sl = slice(cch * 512, (cch + 1) * 512)
                masks = []
                pv = app.tile([F, 512], f32, tag="pv", name="pv")
                for s in range(S):
                    eq_c = wk.tile([1, 512], bf16, tag="eq_c", name="eq_c")
                    nc.vector.tensor_scalar(
                        out=eq_c[:, :], in0=spf[:, sl], scalar1=float(s),
                        scalar2=None, op0=OP.is_equal)
                    mps = mpp.tile([128, 512], f32, tag="mps", name="mps")
                    nc.tensor.matmul(mps[:, :], ones128[:, :], eq_c[:, :],
                                     start=True, stop=True)
                    mask = mkp.tile([128, 512], bf16, tag=f"mask{s}",
                                    name=f"mask{s}")
                    nc.vector.tensor_copy(mask[:, :], mps[:, :])
                    masks.append(mask)

                    ph0 = pp.tile([96, 512], f32, tag="ph0", name="ph0")
                    ph1 = pp.tile([96, 512], f32, tag="ph1", name="ph1")
                    for k in range(3):
                        xm = wk.tile([128, 512], bf16, tag="xm", name="xm")
                        nc.vector.tensor_tensor(out=xm[:, :],
                                                in0=xT[k][:, sl],
                                                in1=mask[:, :], op=OP.mult)
                        nc.tensor.matmul(ph0[:, :], w1t[s][k][:, 0:96],
                                         xm[:, :], start=(k == 0),
                                         stop=(k == 2))
                        nc.tensor.matmul(ph1[:, :], w1t[s][k][:, 96:192],
                                         xm[:, :], start=(k == 0),
                                         stop=(k == 2))
                    h0 = wk.tile([96, 512], bf16, tag="h0", name="h0")
                    h1 = wk.tile([96, 512], bf16, tag="h1", name="h1")
                    nc.scalar.activation(h0[:, :], ph0[:, :],
                                         AF.Gelu_apprx_tanh)
                    nc.scalar.activation(h1[:, :], ph1[:, :],
                                         AF.Gelu_apprx_tanh)
                    nc.tensor.matmul(pv[:, :], w2t[s][0][:, :], h0[:, :],
                                     start=(s == 0), stop=False)
                    nc.tensor.matmul(pv[:, :], w2t[s][1][:, :], h1[:, :],
                                     start=False, stop=(s == S - 1))
                nc.vector.tensor_copy(internalT[:, sl], pv[:, :])

                pn = app.tile([F, 512], f32, tag="pn", name="pn")
                for s in range(S):
                    im = wk.tile([96, 512], bf16, tag="im", name="im")
                    nc.vector.tensor_tensor(out=im[:, :],
                                            in0=internalT[:, sl],
                                            in1=masks[s][0:96, :],
                                            op=OP.mult)
                    nc.tensor.matmul(pn[:, :], wnt[s][:, :], im[:, :],
                                     start=(s == 0), stop=(s == S - 1))
                nc.vector.tensor_copy(nbrT[:, sl], pn[:, :])

                # stage this chunk's nbr rows (transposed back, padded)
                for t in range(4):
                    tg = cch * 4 + t
                    ntp = mpp.tile([128, 96], bf16, tag="ntp", name="ntp")
                    nc.tensor.transpose(ntp[:, :],
                                        nbrT[:, tg * 128:(tg + 1) * 128],
                                        identb[0:96, 0:96])
                    strow = wk.tile([128, 128], bf16, tag="strow",
                                    name="strow")
                    nc.vector.memset(strow[:, 96:128], 0.0)
                    nc.vector.tensor_copy(strow[:, 0:96], ntp[:, :])
                    nc.sync.dma_start(nbr_rows[:, tg, :], strow[:, :])

        tc.strict_bb_all_engine_barrier()

        # ---------------- phase 2: AllGather nbr table + split bounce
        nc.gpsimd.collective_compute(
            "AllGather", mybir.AluOpType.bypass,
            replica_groups=[list(range(NCORE))],
            ins=[nbr_local[:, :]],
            outs=[nbr_full_sh[:, :]],
        )
        BCH = N // 16
        for q in range(16):
            nc.sync.dma_start(nbr_full[q * BCH:(q + 1) * BCH, :],
                              nbr_full_sh[q * BCH:(q + 1) * BCH, :])

        tc.strict_bb_all_engine_barrier()

        # ---------------- phase 3: edge aggregation (quad slots,
        # member-plane-major so the combine is fully contiguous on DVE)
        PB = Tq * 128             # elements per member plane per block
        with tc.tile_pool(name="msg_pool", bufs=3) as mgp, \
             tc.tile_pool(name="edge_work", bufs=3) as ew, \
             tc.tile_pool(name="edge_psum", bufs=2, space="PSUM") as epp:
            for b in range(NBLK_C):
                kqb = b * Tq
                # gathers in ring-sized pieces into one per-block buffer
                msg = mgp.tile([128, QM * PB], bf16, tag="msg", name="msg")
                for q, o0 in enumerate(range(0, QM * PB, 1024)):
                    nidx = min(1024, QM * PB - o0)
                    nc.gpsimd.dma_gather(
                        out_ap=msg[:, o0:o0 + nidx].rearrange(
                            "p (c e) -> p c e", e=128),
                        in_ap=nbr_full[:, :],
                        idxs_ap=idxs[:, kqb * 32 + o0 // 16:
                                     kqb * 32 + (o0 + nidx) // 16],
                        num_idxs=nidx,
                        num_idxs_reg=nidx,
                        elem_size=128,
                        queue_num=(b * NGB + q) % 4)
                # msgw = sum_m member_m * w_m  (weighted quad combine,
                # full 128-wide rows: pad cols are zero in the table)
                acc = ew.tile([128, PB], bf16, tag="acc", name="acc")
                av = acc[:, :].rearrange("p (g f) -> p g f", f=128)
                w0 = wgtb[:, 0 * KQ + kqb:0 * KQ + kqb + Tq]
                nc.vector.tensor_tensor(
                    out=av, in0=msg[:, 0:PB].rearrange(
                        "p (g f) -> p g f", f=128),
                    in1=w0.to_broadcast([128, Tq, 128]), op=OP.mult)
                for m in range(1, QM):
                    tmp = ew.tile([128, PB], bf16, tag="tmp", name="tmp")
                    tv = tmp[:, :].rearrange("p (g f) -> p g f", f=128)
                    wm = wgtb[:, m * KQ + kqb:m * KQ + kqb + Tq]
                    nc.vector.tensor_tensor(
                        out=tv, in0=msg[:, m * PB:(m + 1) * PB].rearrange(
                            "p (g f) -> p g f", f=128),
                        in1=wm.to_broadcast([128, Tq, 128]), op=OP.mult)
                    nc.vector.tensor_tensor(out=acc[:, :], in0=acc[:, :],
                                            in1=tmp[:, :], op=OP.add)
                # one-hot destination tiles
                oh = ew.tile([128, PB], bf16, tag="oh", name="oh")
                ov = oh[:, :].rearrange("p (g f) -> p g f", f=128)
                dl = dlocb[:, kqb:kqb + Tq]
                nc.vector.tensor_tensor(
                    out=ov,
                    in0=iotab[:, :].rearrange("p (g f) -> p g f", f=128),
                    in1=dl.to_broadcast([128, Tq, 128]), op=OP.is_equal)
                pm = epp.tile([F, 128], f32, tag="pm", name="pm")
                for t in range(Tq):
                    nc.tensor.matmul(
                        pm[:, :], acc[:, t * 128:t * 128 + 96],
                        oh[:, t * 128:(t + 1) * 128],
                        start=(t == 0), stop=(t == Tq - 1))
                nc.vector.tensor_copy(mergedT[:, b * 128:(b + 1) * 128],
                                      pm[:, :])

        tc.strict_bb_all_engine_barrier()

        # ---------------- phase 4: final head + charge redistribution
        with tc.tile_pool(name="head_sbuf", bufs=1) as hp, \
             tc.tile_pool(name="head_work", bufs=3) as hw, \
             tc.tile_pool(name="head_psum", bufs=2, space="PSUM") as hpp:
            pre = hp.tile([1, AC], f32, tag="pre", name="pre")
            chg = hp.tile([1, AC], f32, tag="chg", name="chg")

            for cch in range(NCH):
                sl = slice(cch * 512, (cch + 1) * 512)
                pp4 = hpp.tile([S, 512], f32, tag="pp4", name="pp4")
                nc.tensor.matmul(pp4[:, :], wfi[:, :], internalT[:, sl],
                                 start=True, stop=False)
                nc.tensor.matmul(pp4[:, :], wfm[:, :], mergedT[:, sl],
                                 start=False, stop=True)
                sel = hw.tile([S, 512], bf16, tag="sel", name="sel")
                nc.vector.tensor_tensor(out=sel[:, :], in0=pp4[:, :],
                                        in1=eqs[:, sl], op=OP.mult)
                pr1 = hpp.tile([1, 512], f32, tag="pr1", name="pr1")
                nc.tensor.matmul(pr1[:, :], ones4c[:, :], sel[:, :],
                                 start=True, stop=True)
                nc.vector.tensor_copy(pre[:, sl], pr1[:, :])

            # per-molecule redistribution
            msum = hp.tile([1, MOL], f32, tag="msum", name="msum")
            nc.vector.tensor_reduce(
                out=msum[:, :],
                in_=pre[:, :].rearrange("p (m a) -> p m a", a=A),
                axis=mybir.AxisListType.X, op=OP.add)
            adj = hp.tile([1, MOL], f32, tag="adj", name="adj")
            nc.vector.tensor_tensor(out=adj[:, :], in0=tct[:, :],
                                    in1=msum[:, :], op=OP.subtract)
            nc.vector.tensor_scalar(out=adj[:, :], in0=adj[:, :],
                                    scalar1=1.0 / A, scalar2=None,
                                    op0=OP.mult)
            nc.vector.tensor_tensor(
                out=chg[:, :].rearrange("p (m a) -> p m a", a=A),
                in0=pre[:, :].rearrange("p (m a) -> p m a", a=A),
                in1=adj[:, :].to_broadcast([1, MOL, A]),
                op=OP.add)

            nc.sync.dma_start(out_t[0:1, :], chg[:, :])
            nc.sync.dma_start(out_t[1:2, :], pre[:, :])

        for free in reversed(_keep):
            free()

    nc.compile()
    return nc


def _get_nc(Tq, dp2, df2):
    key = (Tq, round(float(dp2), 9), round(float(df2), 9))
    if key not in _CACHE:
        _CACHE[key] = _build(Tq, dp2, df2)
    return _CACHE[key]


# ---------------------------------------------------------------- entry point
def kernel(species, in_features, atom_index12, distances, total_charges,
           W1, W2, Wn, Wf, decay_prefactor, decay_factor, _trace=False):
    import ml_dtypes
    from concourse.bass_utils import run_bass_kernel_spmd

    bf = ml_dtypes.bfloat16
    species = np.asarray(species, np.int32)
    in_features = np.ascontiguousarray(np.asarray(in_features, np.float32))
    atom_index12 = np.asarray(atom_index12, np.int32)
    distances = np.asarray(distances, np.float32)
    total_charges = np.asarray(total_charges, np.float32)
    W1b = np.ascontiguousarray(np.asarray(W1, np.float32).astype(bf))
    W2b = np.ascontiguousarray(np.asarray(W2, np.float32).astype(bf))
    Wnb = np.ascontiguousarray(np.asarray(Wn, np.float32).astype(bf))
    Wfb = np.ascontiguousarray(np.asarray(Wf, np.float32).astype(bf))
    dp2 = float(np.asarray(decay_prefactor)) ** 2
    df2 = float(np.asarray(decay_factor)) ** 2

    Tq, KQ, eidx, dval, dloc = _preprocess_edges(atom_index12, distances)
    nc = _get_nc(Tq, dp2, df2)

    in_maps = []
    for c in range(NCORE):
        in_maps.append({
            "x": np.ascontiguousarray(
                in_features[c * MOL:(c + 1) * MOL].reshape(AC, D).astype(bf)),
            "species": np.ascontiguousarray(
                species[c * MOL:(c + 1) * MOL].reshape(1, AC)),
            "tcharge": np.ascontiguousarray(
                total_charges[c * MOL:(c + 1) * MOL].reshape(1, MOL)),
            "W1": W1b, "W2": W2b, "Wn": Wnb, "Wf": Wfb,
            "eidx": np.ascontiguousarray(eidx[c]),
            "dval": np.ascontiguousarray(dval[c]),
            "dloc": np.ascontiguousarray(dloc[c]),
        })

    res = run_bass_kernel_spmd(nc, in_maps, core_ids=list(range(NCORE)),
                               trace=_trace)
    charges = np.empty((B, A), np.float32)
    precharges = np.empty((B, A), np.float32)
    for c in range(NCORE):
        o = res.results[c]["out"]
        charges[c * MOL:(c + 1) * MOL] = o[0].reshape(MOL, A)
        precharges[c * MOL:(c + 1) * MOL] = o[1].reshape(MOL, A)
    if _trace:
        kernel._last_results = res
    return species.reshape(B, A), charges, precharges
